# revision 11
# baseline (speedup 1.0000x reference)
"""Trainium2 Bass kernel for a transformer decoder block (self-attn + cross-attn + FFN).

Sharding: 8 cores = data-parallel over batch (2) x tensor-parallel over heads (4).
Attention QKV/scores/ctx are head-sharded; out-proj/LayerNorm/FFN are sharded over
query quarters. Cross-core resharding via AllToAll (ctx) and AllGather (x1).

Device layout: activations are kept transposed [d_model, seq] throughout. The host
pre-transposes inputs and re-transposes the output, so the device does zero
transposes. LayerNorm reductions over d (= partitions) are done with M=1 matmuls
against a ones column; broadcasts back over partitions with K=1 matmuls.
"""

import numpy as np

import concourse.bass as bass
import concourse.mybir as mybir
import concourse.tile as tile
from concourse import bacc
from concourse.bass_utils import run_bass_kernel_spmd

F32 = mybir.dt.float32
F32R = mybir.dt.float32r
AF = mybir.ActivationFunctionType
OP = mybir.AluOpType

D = 1024
S = 2048
B = 2
NHEAD = 16
DK = 64
DFF = 4096
NCORES = 8
TP = 4            # tensor-parallel group size (heads)
HPC = NHEAD // TP  # heads per core = 4
DH = HPC * DK      # per-core head dim = 256
Q4 = S // TP       # query quarter = 512
QS = 512           # q stripe for attention
KC = 128           # k chunk
NQS = S // QS      # 4
NKC = S // KC      # 16
NDC = D // 128     # 8
NFC = DFF // 128   # 32
EPS = 1e-5

RG = [[0, 1, 2, 3], [4, 5, 6, 7]]


def _analyze_mask(mask):
    """Per (q-stripe j, k-chunk i) mode: 'free' (all 1), 'skip' (all 0), else
    'mask'. Also returns the masked tiles, transposed to [k, q], as an array
    [n_tiles, 128, 512] plus the (j, i) -> tile index map."""
    mt = mask.T  # [k, q]
    modes = [[None] * NKC for _ in range(NQS)]
    tiles = []
    index = {}
    for j in range(NQS):
        for i in range(NKC):
            sub = mt[i * KC:(i + 1) * KC, j * QS:(j + 1) * QS]
            if sub.all():
                modes[j][i] = 'free'
            elif not sub.any():
                modes[j][i] = 'skip'
            else:
                modes[j][i] = 'mask'
                index[(j, i)] = len(tiles)
                tiles.append(np.ascontiguousarray(sub, dtype=np.float32))
    tiles = np.stack(tiles) if tiles else np.zeros((1, KC, QS), np.float32)
    return modes, tiles, index


def _attention(nc, tc, pools, QT, KT, Vaug, modes, mask_index, mask_d, cc_in,
               ones_sb):
    """Head-TP attention in transposed layout. QT/KT: [128, 2, 2048] f32r
    (head h lives at partitions 64*(h%2).. of pchunk h//2). Vaug:
    [128, 16, 4, 65] f32r with ones in col 64. Writes normalized ctxT [64, 512]
    blocks to cc_in[j, 64h:64h+64, :]."""
    sb = pools['attn_sb']
    ps = pools['attn_ps']
    for j in range(NQS):
        mtiles = {}
        for i in range(NKC):
            if modes[j][i] == 'mask':
                mtile = sb.tile([KC, QS], F32R, name=f"mtile{len(mtiles) % 4}",
                                tag=f"mtile{len(mtiles) % 4}", bufs=2)
                nc.sync.dma_start(mtile[:], mask_d[mask_index[(j, i)], :, :])
                mtiles[i] = mtile
        live = [i for i in range(NKC) if modes[j][i] != 'skip']
        first_i, last_i = live[0], live[-1]
        for hp in range(2):
            psC = [ps.tile([65, QS], F32, name=f"psC{s}", tag=f"psC{s}", bufs=1)
                   for s in range(2)]
            for i in live:
                for s in range(2):
                    h = 2 * hp + s
                    pb = 64 * s
                    psS = ps.tile([KC, QS], F32, name=f"psS{s}", tag=f"psS{s}",
                                  bufs=2)
                    nc.tensor.matmul(
                        psS[:],
                        KT[pb:pb + 64, hp, i * KC:(i + 1) * KC],
                        QT[pb:pb + 64, hp, j * QS:(j + 1) * QS],
                        start=True, stop=True)
                    E = sb.tile([KC, QS], F32R, name=f"E{s}", tag=f"E{s}",
                                bufs=3)
                    nc.scalar.activation(E[:], psS[:], AF.Exp)
                    if i in mtiles:
                        nc.vector.tensor_mul(E[:], E[:], mtiles[i][:])
                    nc.tensor.matmul(
                        psC[s][:], Vaug[:, i, h, :], E[:],
                        start=(i == first_i), stop=(i == last_i))
            for s in range(2):
                h = 2 * hp + s
                rec = sb.tile([1, QS], F32R, name=f"rec{s}", tag=f"rec{s}",
                              bufs=2)
                with nc.allow_low_precision(reason="softmax denom recip"):
                    nc.vector.reciprocal(rec[:], psC[s][64:65, :])
                psB = ps.tile([64, QS], F32, name="psB", tag="psB", bufs=2)
                nc.tensor.matmul(psB[:], ones_sb[0:1, 0:64], rec[:],
                                 start=True, stop=True)
                rb = sb.tile([64, QS], F32, name=f"rb{s}", tag=f"rb{s}",
                             bufs=2)
                nc.scalar.copy(rb[:], psB[:])
                ctx = sb.tile([64, QS], F32R, name=f"ctx{s}", tag=f"ctx{s}",
                              bufs=2)
                nc.vector.tensor_mul(ctx[:], psC[s][0:64, :], rb[:])
                nc.sync.dma_start(
                    cc_in[64 * h:64 * h + 64, j * QS:(j + 1) * QS], ctx[:])


def _qkvT_proj(nc, pools, xT, w_sb, b_sb, outT, pool_tag):
    """outT[:, pair, stripe] = w.T @ xT + b for 2 dout pairs x 4 stripes."""
    ps = pools['proj_ps']
    for pair in range(2):
        for jq in range(NQS):
            psq = ps.tile([128, QS], F32, name=f"psq_{pool_tag}",
                          tag=f"psq_{pool_tag}", bufs=2)
            for dc in range(NDC):
                nc.tensor.matmul(
                    psq[:],
                    w_sb[:, dc, pair * 128:(pair + 1) * 128],
                    xT[:, dc, jq * QS:(jq + 1) * QS],
                    start=(dc == 0), stop=(dc == NDC - 1))
            nc.scalar.activation(outT[:, pair, jq * QS:(jq + 1) * QS], psq[:],
                                 AF.Identity, bias=b_sb[:, pair:pair + 1])


def _v_proj(nc, pools, xT, wv_sb, bv_bc, Vaug, pool_tag):
    """Vaug[:, sb_i, h, 0:64] = (xT.T @ wv + bv) natural layout, 16 s-blocks."""
    ps = pools['proj_ps']
    for sb_i in range(NKC):
        psv = ps.tile([128, DH], F32, name=f"psv_{pool_tag}",
                      tag=f"psv_{pool_tag}", bufs=2)
        for dc in range(NDC):
            nc.tensor.matmul(
                psv[:],
                xT[:, dc, sb_i * KC:(sb_i + 1) * KC],
                wv_sb[:, dc, :],
                start=(dc == 0), stop=(dc == NDC - 1))
        nc.vector.tensor_tensor(
            out=Vaug[:, sb_i, :, 0:64],
            in0=psv[:].rearrange("p (h d) -> p h d", h=HPC),
            in1=bv_bc[:].rearrange("p (h d) -> p h d", h=HPC),
            op=OP.add)


def _layernorm_T(nc, pools, xraw, g_sb, b_sb, outT, ones_col, ones_sb, eps_sb,
                 pool_tag):
    """LayerNorm over d (partition-chunked) in transposed layout.
    xraw/outT: [128, 8, 512]. Stats via M=1 matmuls, broadcast via K=1."""
    sb = pools['ln_sb']
    ps = pools['ln_ps']
    pssum = ps.tile([1, Q4], F32, name=f"pssum_{pool_tag}", tag="pssum", bufs=1)
    pssq = ps.tile([1, Q4], F32, name=f"pssq_{pool_tag}", tag="pssq", bufs=1)
    for m in range(NDC):
        nc.tensor.matmul(pssum[:], ones_col[:, 0:1], xraw[:, m, :],
                         start=(m == 0), stop=(m == NDC - 1))
    for m in range(NDC):
        xsq = sb.tile([128, Q4], F32R, name="xsq", tag="xsq", bufs=2)
        nc.vector.tensor_mul(xsq[:], xraw[:, m, :], xraw[:, m, :])
        nc.tensor.matmul(pssq[:], ones_col[:, 0:1], xsq[:],
                         start=(m == 0), stop=(m == NDC - 1))
    mu = sb.tile([1, Q4], F32R, name="mu", tag="mu", bufs=2)
    nc.scalar.activation(mu[:], pssum[:], AF.Copy, scale=1.0 / D)
    msq = sb.tile([1, Q4], F32, name="msq", tag="msq", bufs=2)
    nc.scalar.activation(msq[:], pssq[:], AF.Copy, scale=1.0 / D)
    musq = sb.tile([1, Q4], F32, name="musq", tag="musq", bufs=2)
    nc.vector.tensor_mul(musq[:], mu[:], mu[:])
    var = sb.tile([1, Q4], F32, name="var", tag="var", bufs=2)
    nc.vector.tensor_tensor(out=var[:], in0=msq[:], in1=musq[:], op=OP.subtract)
    sd = sb.tile([1, Q4], F32, name="sd", tag="sd", bufs=2)
    nc.scalar.activation(sd[:], var[:], AF.Sqrt, bias=eps_sb[:])
    rstd = sb.tile([1, Q4], F32R, name="rstd", tag="rstd", bufs=2)
    with nc.allow_low_precision(reason="LN rstd recip"):
        nc.vector.reciprocal(rstd[:], sd[:])
    psmu = ps.tile([128, Q4], F32, name=f"psmu_{pool_tag}", tag="psmu", bufs=1)
    psrs = ps.tile([128, Q4], F32, name=f"psrs_{pool_tag}", tag="psrs", bufs=1)
    nc.tensor.matmul(psmu[:], ones_sb[0:1, :], mu[:], start=True, stop=True)
    nc.tensor.matmul(psrs[:], ones_sb[0:1, :], rstd[:], start=True, stop=True)
    mu_b = sb.tile([128, Q4], F32, name="mu_b", tag="mu_b", bufs=2)
    rs_b = sb.tile([128, Q4], F32, name="rs_b", tag="rs_b", bufs=2)
    nc.scalar.copy(mu_b[:], psmu[:])
    nc.scalar.copy(rs_b[:], psrs[:])
    for m in range(NDC):
        tmp = sb.tile([128, Q4], F32, name="lntmp", tag="lntmp", bufs=2)
        nc.vector.tensor_tensor(out=tmp[:], in0=xraw[:, m, :], in1=mu_b[:],
                                op=OP.subtract)
        nc.vector.tensor_mul(tmp[:], tmp[:], rs_b[:])
        nc.vector.tensor_scalar(
            out=outT[:, m, :], in0=tmp[:],
            scalar1=g_sb[:, m:m + 1], scalar2=b_sb[:, m:m + 1],
            op0=OP.mult, op1=OP.add)


def build(modes_sa, n_mask_sa, modes_ca, n_mask_ca, mask_index_sa,
          mask_index_ca):
    nc = bacc.Bacc("TRN2", num_devices=NCORES)

    # ---- DRAM I/O (all f32 bits; f32r where the PE consumes it) ----
    din = {}
    def dram_in(name, shape, dt=F32R):
        din[name] = nc.dram_tensor(name, shape, dt, kind="ExternalInput")
        return din[name]

    tgtT_d = dram_in("tgtT", [D, S])
    tgtTq_d = dram_in("tgtTq", [D, Q4], F32)
    memT_d = dram_in("memT", [D, S])
    wq_sa_d = dram_in("wq_sa", [D, DH])
    wk_sa_d = dram_in("wk_sa", [D, DH])
    wv_sa_d = dram_in("wv_sa", [D, DH])
    wo_sa_d = dram_in("wo_sa", [D, D])
    wq_ca_d = dram_in("wq_ca", [D, DH])
    wk_ca_d = dram_in("wk_ca", [D, DH])
    wv_ca_d = dram_in("wv_ca", [D, DH])
    wo_ca_d = dram_in("wo_ca", [D, D])
    w1_d = dram_in("w1", [D, DFF])
    w2_d = dram_in("w2", [DFF, D])
    bq_sa_d = dram_in("bq_sa", [DH], F32)
    bk_sa_d = dram_in("bk_sa", [DH], F32)
    bv_sa_d = dram_in("bv_sa", [DH], F32)
    bo_sa_d = dram_in("bo_sa", [D], F32)
    bq_ca_d = dram_in("bq_ca", [DH], F32)
    bk_ca_d = dram_in("bk_ca", [DH], F32)
    bv_ca_d = dram_in("bv_ca", [DH], F32)
    bo_ca_d = dram_in("bo_ca", [D], F32)
    b1_d = dram_in("b1", [DFF], F32)
    b2_d = dram_in("b2", [D], F32)
    ln_d = {}
    for i in (1, 2, 3):
        ln_d[f"g{i}"] = dram_in(f"ln{i}_g", [D], F32)
        ln_d[f"b{i}"] = dram_in(f"ln{i}_b", [D], F32)
    ones_d = dram_in("ones", [128, 128])
    mask_sa_d = dram_in("mask_sa", [max(n_mask_sa, 1), KC, QS])
    mask_ca_d = dram_in("mask_ca", [max(n_mask_ca, 1), KC, QS])
    out_d = nc.dram_tensor("out", [D, Q4], F32, kind="ExternalOutput")

    with tile.TileContext(nc) as tc:
        with (
            tc.tile_pool(name="persist", bufs=1) as persist,
            tc.tile_pool(name="dram", bufs=1, space="DRAM") as dram,
        ):
            # ---- collective scratch ----
            cc_in_sa = dram.tile([DH, S], F32R)
            cc_out_sa = dram.tile([TP * DH, S], F32R)
            cc_in_x1 = dram.tile([D, Q4], F32R)
            cc_out_x1 = dram.tile([TP * D, Q4], F32R)
            cc_in_ca = dram.tile([DH, S], F32R)
            cc_out_ca = dram.tile([TP * DH, S], F32R)

            # my q-quarter offset (runtime, from the SPMD partition id)
            qoff = (nc.sync.partition_id() % TP) * Q4

            # ---- small persistent constants ----
            ones_sb = persist.tile([1, 128], F32R)
            nc.sync.dma_start(ones_sb[:], ones_d[0:1, :])
            ones_col = persist.tile([128, 1], F32R)
            nc.sync.dma_start(ones_col[:], ones_d[:, 0:1])
            eps_sb = persist.tile([1, 1], F32)
            nc.vector.memset(eps_sb[:], EPS)

            def vec_sb(d, n):
                t = persist.tile([128, n // 128], F32,
                                 name=f"v_{d.name}", tag=f"v_{d.name}")
                nc.sync.dma_start(t[:], d[:].rearrange("(c p) -> p c", p=128))
                return t

            bq_sa_sb = vec_sb(bq_sa_d, DH)
            bk_sa_sb = vec_sb(bk_sa_d, DH)
            bo_sa_sb = vec_sb(bo_sa_d, D)
            bq_ca_sb = vec_sb(bq_ca_d, DH)
            bk_ca_sb = vec_sb(bk_ca_d, DH)
            bo_ca_sb = vec_sb(bo_ca_d, D)
            b1_sb = vec_sb(b1_d, DFF)
            b2_sb = vec_sb(b2_d, D)
            ln_sb = {k: vec_sb(v, D) for k, v in ln_d.items()}

            def bcast_sb(d, n):
                t = persist.tile([128, n], F32, name=f"bc_{d.name}",
                                 tag=f"bc_{d.name}")
                nc.gpsimd.dma_start(
                    out=t[:],
                    in_=bass.AP(tensor=d, offset=0, ap=[[0, 128], [1, n]]))
                return t

            bv_sa_bc = bcast_sb(bv_sa_d, DH)
            bv_ca_bc = bcast_sb(bv_ca_d, DH)

            pools = {}

            # ================= Phase 1: self-attn QKV =================
            with (
                tc.tile_pool(name="p1_w", bufs=1) as p1w,
                tc.tile_pool(name="p1_x", bufs=1) as p1x,
                tc.tile_pool(name="attn_sa_io", bufs=1) as sa_io,
            ):
                tgtT = p1x.tile([128, NDC, S], F32R)
                nc.sync.dma_start(
                    tgtT[:], tgtT_d[:].rearrange("(c p) s -> p c s", p=128))

                def load_w(d, cols):
                    t = p1w.tile([128, NDC, cols], F32R, name=f"w_{d.name}",
                                 tag=f"w_{d.name}")
                    nc.sync.dma_start(
                        t[:], d[:].rearrange("(c p) n -> p c n", p=128))
                    return t

                wq_sb = load_w(wq_sa_d, DH)
                wk_sb = load_w(wk_sa_d, DH)
                wv_sb = load_w(wv_sa_d, DH)

                QT_sa = sa_io.tile([128, 2, S], F32R)
                KT_sa = sa_io.tile([128, 2, S], F32R)
                Vaug_sa = sa_io.tile([128, NKC, HPC, 65], F32R)
                nc.sync.dma_start(
                    Vaug_sa[:, :, :, 64:65],
                    ones_d[:, 0:64].rearrange("p (a b c) -> p a b c",
                                              a=NKC, b=HPC))

                with tc.tile_pool(name="proj_ps", bufs=1,
                                  space="PSUM") as proj_ps:
                    pools['proj_ps'] = proj_ps
                    _qkvT_proj(nc, pools, tgtT, wq_sb, bq_sa_sb, QT_sa, "qsa")
                    _qkvT_proj(nc, pools, tgtT, wk_sb, bk_sa_sb, KT_sa, "ksa")
                    _v_proj(nc, pools, tgtT, wv_sb, bv_sa_bc, Vaug_sa, "vsa")

                # ============ Phase 2: self attention ============
                with (
                    tc.tile_pool(name="attn_sb", bufs=1) as attn_sb,
                    tc.tile_pool(name="attn_ps", bufs=1, space="PSUM") as attn_ps,
                ):
                    pools['attn_sb'] = attn_sb
                    pools['attn_ps'] = attn_ps
                    _attention(nc, tc, pools, QT_sa, KT_sa, Vaug_sa, modes_sa,
                               mask_index_sa, mask_sa_d, cc_in_sa, ones_sb)

            nc.gpsimd.collective_compute(
                "AllGather", OP.bypass, replica_groups=RG,
                ins=[cc_in_sa[:].opt()], outs=[cc_out_sa[:].opt()])

            # ===== Phase 3: cross-attn K/V (overlaps the AllToAll) =====
            with (
                tc.tile_pool(name="p3_w", bufs=1) as p3w,
                tc.tile_pool(name="attn_ca_io", bufs=1) as ca_io,
            ):
                def load_w3(d, cols):
                    t = p3w.tile([128, NDC, cols], F32R, name=f"w_{d.name}",
                                 tag=f"w_{d.name}")
                    nc.sync.dma_start(
                        t[:], d[:].rearrange("(c p) n -> p c n", p=128))
                    return t

                KT_ca = ca_io.tile([128, 2, S], F32R)
                Vaug_ca = ca_io.tile([128, NKC, HPC, 65], F32R)
                nc.sync.dma_start(
                    Vaug_ca[:, :, :, 64:65],
                    ones_d[:, 0:64].rearrange("p (a b c) -> p a b c",
                                              a=NKC, b=HPC))
                with (
                    tc.tile_pool(name="p3_x", bufs=1) as p3x,
                    tc.tile_pool(name="proj_ps2", bufs=1, space="PSUM") as proj_ps2,
                ):
                    pools['proj_ps'] = proj_ps2
                    memT = p3x.tile([128, NDC, S], F32R)
                    nc.sync.dma_start(
                        memT[:], memT_d[:].rearrange("(c p) s -> p c s", p=128))
                    wk_ca_sb = load_w3(wk_ca_d, DH)
                    wv_ca_sb = load_w3(wv_ca_d, DH)
                    _qkvT_proj(nc, pools, memT, wk_ca_sb, bk_ca_sb, KT_ca, "kca")
                    _v_proj(nc, pools, memT, wv_ca_sb, bv_ca_bc, Vaug_ca, "vca")

                # ====== Phase 4: self out-proj + residual + LN1 ======
                with (
                    tc.tile_pool(name="p4_sb", bufs=1) as p4sb,
                    tc.tile_pool(name="ln_sb", bufs=1) as ln_sb_pool,
                    tc.tile_pool(name="x1_keep", bufs=1) as x1_keep,
                    tc.tile_pool(name="p4_ps", bufs=1, space="PSUM") as p4ps,
                    tc.tile_pool(name="ln_ps", bufs=1, space="PSUM") as ln_ps,
                ):
                    pools['ln_sb'] = ln_sb_pool
                    pools['ln_ps'] = ln_ps
                    gctx = p4sb.tile([128, NDC, Q4], F32R)
                    for c in range(NDC):
                        nc.sync.dma_start(
                            gctx[:, c, :],
                            cc_out_sa[128 * c:128 * (c + 1), bass.ds(qoff, Q4)])
                    x1raw = p4sb.tile([128, NDC, Q4], F32R)
                    for m in range(NDC):
                        wom = p4sb.tile([128, NDC, 128], F32R, name="wom",
                                        tag="wom", bufs=2)
                        nc.sync.dma_start(
                            wom[:],
                            wo_sa_d[:, m * 128:(m + 1) * 128]
                            .rearrange("(c p) n -> p c n", p=128))
                        tqm = p4sb.tile([128, Q4], F32, name="tqm", tag="tqm",
                                        bufs=2)
                        nc.sync.dma_start(
                            tqm[:], tgtTq_d[m * 128:(m + 1) * 128, :])
                        pso = p4ps.tile([128, Q4], F32, name="pso", tag="pso",
                                        bufs=2)
                        for c in range(NDC):
                            nc.tensor.matmul(
                                pso[:], wom[:, c, :], gctx[:, c, :],
                                start=(c == 0), stop=(c == NDC - 1))
                        t_sb = p4sb.tile([128, Q4], F32, name="t_sb",
                                         tag="t_sb", bufs=2)
                        nc.scalar.activation(t_sb[:], pso[:], AF.Identity,
                                             bias=bo_sa_sb[:, m:m + 1])
                        nc.vector.tensor_tensor(out=x1raw[:, m, :], in0=t_sb[:],
                                                in1=tqm[:], op=OP.add)

                    x1T = x1_keep.tile([128, NDC, Q4], F32R)
                    _layernorm_T(nc, pools, x1raw, ln_sb["g1"], ln_sb["b1"],
                                 x1T, ones_col, ones_sb, eps_sb, "ln1")
                    nc.sync.dma_start(
                        cc_in_x1[:].rearrange("(c p) q -> p c q", p=128), x1T[:])

                nc.gpsimd.collective_compute(
                    "AllGather", OP.bypass, replica_groups=RG,
                    ins=[cc_in_x1[:].opt()], outs=[cc_out_x1[:].opt()])

                # ========= Phase 5: cross-attn Q projection =========
                QT_ca = ca_io.tile([128, 2, S], F32R)
                with (
                    tc.tile_pool(name="p5_sb", bufs=2) as p5sb,
                    tc.tile_pool(name="proj_ps3", bufs=1, space="PSUM") as proj_ps3,
                ):
                    wq_ca_sb = load_w3(wq_ca_d, DH)
                    for jq in range(NQS):
                        x1f = p5sb.tile([128, NDC, QS], F32R, name="x1f",
                                        tag="x1f")
                        nc.sync.dma_start(
                            x1f[:],
                            cc_out_x1[D * jq:D * (jq + 1), :]
                            .rearrange("(c p) q -> p c q", p=128))
                        for pair in range(2):
                            psq = proj_ps3.tile([128, QS], F32, name="psq5",
                                                tag="psq5", bufs=2)
                            for dc in range(NDC):
                                nc.tensor.matmul(
                                    psq[:],
                                    wq_ca_sb[:, dc, pair * 128:(pair + 1) * 128],
                                    x1f[:, dc, :],
                                    start=(dc == 0), stop=(dc == NDC - 1))
                            nc.scalar.activation(
                                QT_ca[:, pair, jq * QS:(jq + 1) * QS], psq[:],
                                AF.Identity, bias=bq_ca_sb[:, pair:pair + 1])

                # ============ Phase 6: cross attention ============
                with (
                    tc.tile_pool(name="attn_sb2", bufs=1) as attn_sb2,
                    tc.tile_pool(name="attn_ps2", bufs=1, space="PSUM") as attn_ps2,
                ):
                    pools['attn_sb'] = attn_sb2
                    pools['attn_ps'] = attn_ps2
                    _attention(nc, tc, pools, QT_ca, KT_ca, Vaug_ca, modes_ca,
                               mask_index_ca, mask_ca_d, cc_in_ca, ones_sb)

            nc.gpsimd.collective_compute(
                "AllGather", OP.bypass, replica_groups=RG,
                ins=[cc_in_ca[:].opt()], outs=[cc_out_ca[:].opt()])

            # ====== Phase 7: cross out-proj + residual + LN2 ======
            x2T_pool = tc.alloc_tile_pool(name="x2_keep", bufs=1)
            with (
                tc.tile_pool(name="p7_sb", bufs=1) as p7sb,
                tc.tile_pool(name="ln_sb2", bufs=1) as ln_sb2,
                tc.tile_pool(name="p7_ps", bufs=1, space="PSUM") as p7ps,
                tc.tile_pool(name="ln_ps2", bufs=1, space="PSUM") as ln_ps2,
            ):
                pools['ln_sb'] = ln_sb2
                pools['ln_ps'] = ln_ps2
                gctx2 = p7sb.tile([128, NDC, Q4], F32R)
                for c in range(NDC):
                    nc.sync.dma_start(
                        gctx2[:, c, :],
                        cc_out_ca[128 * c:128 * (c + 1), bass.ds(qoff, Q4)])
                x2raw = p7sb.tile([128, NDC, Q4], F32R)
                for m in range(NDC):
                    wom2 = p7sb.tile([128, NDC, 128], F32R, name="wom2",
                                     tag="wom2", bufs=2)
                    nc.sync.dma_start(
                        wom2[:],
                        wo_ca_d[:, m * 128:(m + 1) * 128]
                        .rearrange("(c p) n -> p c n", p=128))
                    x1qm = p7sb.tile([128, Q4], F32R, name="x1qm", tag="x1qm",
                                     bufs=2)
                    nc.sync.dma_start(
                        x1qm[:], cc_in_x1[m * 128:(m + 1) * 128, :])
                    pso = p7ps.tile([128, Q4], F32, name="pso7", tag="pso7",
                                    bufs=2)
                    for c in range(NDC):
                        nc.tensor.matmul(
                            pso[:], wom2[:, c, :], gctx2[:, c, :],
                            start=(c == 0), stop=(c == NDC - 1))
                    t_sb = p7sb.tile([128, Q4], F32, name="t_sb7", tag="t_sb7",
                                     bufs=2)
                    nc.scalar.activation(t_sb[:], pso[:], AF.Identity,
                                         bias=bo_ca_sb[:, m:m + 1])
                    nc.vector.tensor_tensor(out=x2raw[:, m, :], in0=t_sb[:],
                                            in1=x1qm[:], op=OP.add)

                x2T = x2T_pool.tile([128, NDC, Q4], F32R)
                _layernorm_T(nc, pools, x2raw, ln_sb["g2"], ln_sb["b2"],
                             x2T, ones_col, ones_sb, eps_sb, "ln2")

            # ================= Phase 8: FFN + LN3 =================
            with (
                tc.tile_pool(name="p8_h", bufs=1) as p8h,
                tc.tile_pool(name="p8_w", bufs=3) as p8w,
                tc.tile_pool(name="p8_sb", bufs=1) as p8sb,
                tc.tile_pool(name="ln_sb3", bufs=1) as ln_sb3,
                tc.tile_pool(name="p8_ps", bufs=1, space="PSUM") as p8ps,
                tc.tile_pool(name="ln_ps3", bufs=1, space="PSUM") as ln_ps3,
            ):
                pools['ln_sb'] = ln_sb3
                pools['ln_ps'] = ln_ps3
                hT = p8h.tile([128, NFC, Q4], F32R)
                for f in range(NFC):
                    w1f = p8w.tile([128, NDC, 128], F32R, name="w1f", tag="w1f")
                    nc.sync.dma_start(
                        w1f[:],
                        w1_d[:, f * 128:(f + 1) * 128]
                        .rearrange("(c p) n -> p c n", p=128))
                    psh = p8ps.tile([128, Q4], F32, name="psh", tag="psh",
                                    bufs=2)
                    for m in range(NDC):
                        nc.tensor.matmul(psh[:], w1f[:, m, :], x2T[:, m, :],
                                         start=(m == 0), stop=(m == NDC - 1))
                    nc.scalar.activation(hT[:, f, :], psh[:], AF.Relu,
                                         bias=b1_sb[:, f:f + 1])
                x3raw = p8sb.tile([128, NDC, Q4], F32R)
                for m in range(NDC):
                    w2m = p8w.tile([128, NFC, 128], F32R, name="w2m", tag="w2m",
                                   bufs=2)
                    nc.sync.dma_start(
                        w2m[:],
                        w2_d[:, m * 128:(m + 1) * 128]
                        .rearrange("(c p) n -> p c n", p=128))
                    psf = p8ps.tile([128, Q4], F32, name="psf", tag="psf",
                                    bufs=2)
                    for f in range(NFC):
                        nc.tensor.matmul(psf[:], w2m[:, f, :], hT[:, f, :],
                                         start=(f == 0), stop=(f == NFC - 1))
                    t_sb = p8sb.tile([128, Q4], F32, name="t_sb8", tag="t_sb8",
                                     bufs=2)
                    nc.scalar.activation(t_sb[:], psf[:], AF.Identity,
                                         bias=b2_sb[:, m:m + 1])
                    nc.vector.tensor_tensor(out=x3raw[:, m, :], in0=t_sb[:],
                                            in1=x2T[:, m, :], op=OP.add)

                x3T = p8sb.tile([128, NDC, Q4], F32)
                _layernorm_T(nc, pools, x3raw, ln_sb["g3"], ln_sb["b3"],
                             x3T, ones_col, ones_sb, eps_sb, "ln3")
                nc.sync.dma_start(
                    out_d[:].rearrange("(c p) q -> p c q", p=128), x3T[:])
            x2T_pool.release()

    nc.finalize()
    return nc


_CACHE = {}


def _get_kernel(tgt_mask, memory_mask):
    modes_sa, tiles_sa, idx_sa = _analyze_mask(np.asarray(tgt_mask))
    modes_ca, tiles_ca, idx_ca = _analyze_mask(np.asarray(memory_mask))
    key = (tuple(map(tuple, modes_sa)), tuple(map(tuple, modes_ca)))
    if key not in _CACHE:
        nc = build(modes_sa, len(idx_sa), modes_ca, len(idx_ca), idx_sa, idx_ca)
        _CACHE[key] = nc
    return _CACHE[key], tiles_sa, tiles_ca


def _run(inputs, trace=False):
    tgt = np.asarray(inputs["tgt"], np.float32)
    memory = np.asarray(inputs["memory"], np.float32)
    nc, tiles_sa, tiles_ca = _get_kernel(inputs["tgt_mask"],
                                         inputs["memory_mask"])

    f32 = lambda x: np.ascontiguousarray(np.asarray(x), dtype=np.float32)
    ones128 = np.ones((128, 128), np.float32)
    shared = {
        "wo_sa": f32(inputs["sa_wo"]), "bo_sa": f32(inputs["sa_bo"]),
        "wo_ca": f32(inputs["ca_wo"]), "bo_ca": f32(inputs["ca_bo"]),
        "w1": f32(inputs["ff_w1"]), "b1": f32(inputs["ff_b1"]),
        "w2": f32(inputs["ff_w2"]), "b2": f32(inputs["ff_b2"]),
        "ln1_g": f32(inputs["ln1_g"]), "ln1_b": f32(inputs["ln1_b"]),
        "ln2_g": f32(inputs["ln2_g"]), "ln2_b": f32(inputs["ln2_b"]),
        "ln3_g": f32(inputs["ln3_g"]), "ln3_b": f32(inputs["ln3_b"]),
        "ones": ones128, "mask_sa": tiles_sa, "mask_ca": tiles_ca,
    }
    scale = 1.0 / np.sqrt(DK)
    in_maps = []
    for cid in range(NCORES):
        b, g = cid // TP, cid % TP
        hs = slice(g * DH, (g + 1) * DH)
        m = dict(shared)
        m["tgtT"] = f32(tgt[:, b, :].T)
        m["tgtTq"] = f32(tgt[g * Q4:(g + 1) * Q4, b, :].T)
        m["memT"] = f32(memory[:, b, :].T)
        m["wq_sa"] = f32(np.asarray(inputs["sa_wq"])[:, hs] * scale)
        m["bq_sa"] = f32(np.asarray(inputs["sa_bq"])[hs] * scale)
        m["wk_sa"] = f32(np.asarray(inputs["sa_wk"])[:, hs])
        m["bk_sa"] = f32(np.asarray(inputs["sa_bk"])[hs])
        m["wv_sa"] = f32(np.asarray(inputs["sa_wv"])[:, hs])
        m["bv_sa"] = f32(np.asarray(inputs["sa_bv"])[hs])
        m["wq_ca"] = f32(np.asarray(inputs["ca_wq"])[:, hs] * scale)
        m["bq_ca"] = f32(np.asarray(inputs["ca_bq"])[hs] * scale)
        m["wk_ca"] = f32(np.asarray(inputs["ca_wk"])[:, hs])
        m["bk_ca"] = f32(np.asarray(inputs["ca_bk"])[hs])
        m["wv_ca"] = f32(np.asarray(inputs["ca_wv"])[:, hs])
        m["bv_ca"] = f32(np.asarray(inputs["ca_bv"])[hs])
        in_maps.append(m)

    res = run_bass_kernel_spmd(nc, in_maps, core_ids=list(range(NCORES)),
                               trace=trace)
    out = np.empty((S, B, D), np.float32)
    for cid in range(NCORES):
        b, g = cid // TP, cid % TP
        out[g * Q4:(g + 1) * Q4, b, :] = res.results[cid]["out"].T
    return out, res


def kernel(**inputs):
    out, _ = _run(inputs, trace=False)
    return out


# revision 15
# speedup vs baseline: 1.0801x; 1.0801x over previous
"""Trainium2 Bass kernel for a transformer decoder block (self-attn + cross-attn + FFN).

Sharding: 8 cores = data-parallel over batch (2) x tensor-parallel over heads (4).
Attention QKV/scores/ctx are head-sharded; out-proj/LayerNorm/FFN are sharded over
query quarters. Cross-core resharding via AllToAll (ctx) and AllGather (x1).

Device layout: activations are kept transposed [d_model, seq] throughout. The host
pre-transposes inputs and re-transposes the output, so the device does zero
transposes. LayerNorm reductions over d (= partitions) are done with M=1 matmuls
against a ones column; broadcasts back over partitions with K=1 matmuls.
"""

import numpy as np

import concourse.bass as bass
import concourse.mybir as mybir
import concourse.tile as tile
from concourse import bacc
from concourse.bass_utils import run_bass_kernel_spmd

F32 = mybir.dt.float32
F32R = mybir.dt.float32r
AF = mybir.ActivationFunctionType
OP = mybir.AluOpType

D = 1024
S = 2048
B = 2
NHEAD = 16
DK = 64
DFF = 4096
NCORES = 8
TP = 4            # tensor-parallel group size (heads)
HPC = NHEAD // TP  # heads per core = 4
DH = HPC * DK      # per-core head dim = 256
Q4 = S // TP       # query quarter = 512
QS = 512           # q stripe for attention
KC = 128           # k chunk
NQS = S // QS      # 4
NKC = S // KC      # 16
NDC = D // 128     # 8
NFC = DFF // 128   # 32
EPS = 1e-5

RG = [[0, 1, 2, 3], [4, 5, 6, 7]]


def _analyze_mask(mask):
    """Per (q-stripe j, k-chunk i) mode: 'free' (all 1), 'skip' (all 0), else
    'mask'. Also returns the masked tiles, transposed to [k, q], as an array
    [n_tiles, 128, 512] plus the (j, i) -> tile index map."""
    mt = mask.T  # [k, q]
    modes = [[None] * NKC for _ in range(NQS)]
    tiles = []
    index = {}
    for j in range(NQS):
        for i in range(NKC):
            sub = mt[i * KC:(i + 1) * KC, j * QS:(j + 1) * QS]
            if sub.all():
                modes[j][i] = 'free'
            elif not sub.any():
                modes[j][i] = 'skip'
            else:
                modes[j][i] = 'mask'
                index[(j, i)] = len(tiles)
                tiles.append(np.ascontiguousarray(sub, dtype=np.float32))
    tiles = np.stack(tiles) if tiles else np.zeros((1, KC, QS), np.float32)
    return modes, tiles, index


def _attention(nc, tc, pools, QT, KT, Vaug, modes, mask_index, mask_d, cc_in,
               ones_sb):
    """Head-TP attention in transposed layout. QT/KT: [128, 2, 2048] f32r
    (head h lives at partitions 64*(h%2).. of pchunk h//2). Vaug:
    [128, 16, 4, 65] f32r with ones in col 64. Writes normalized ctxT [64, 512]
    blocks to cc_in[j, 64h:64h+64, :]."""
    sb = pools['attn_sb']
    ps = pools['attn_ps']
    for j in range(NQS):
        mtiles = {}
        for i in range(NKC):
            if modes[j][i] == 'mask':
                mtile = sb.tile([KC, QS], F32R, name=f"mtile{len(mtiles) % 4}",
                                tag=f"mtile{len(mtiles) % 4}", bufs=2)
                nc.sync.dma_start(mtile[:], mask_d[mask_index[(j, i)], :, :])
                mtiles[i] = mtile
        live = [i for i in range(NKC) if modes[j][i] != 'skip']
        first_i, last_i = live[0], live[-1]
        for hp in range(2):
            psC = [ps.tile([65, QS], F32, name=f"psC{s}", tag=f"psC{s}", bufs=2)
                   for s in range(2)]
            # software-pipelined: ctx matmul for chunk i-1 is emitted after the
            # score matmuls for chunk i, so the in-order PE never waits on exp
            pend = []  # (s, h, i, E)
            for i in live:
                for s in range(2):
                    h = 2 * hp + s
                    pb = 64 * s
                    psS = ps.tile([KC, QS], F32, name=f"psS{s}", tag=f"psS{s}",
                                  bufs=2)
                    nc.tensor.matmul(
                        psS[:],
                        KT[pb:pb + 64, hp, i * KC:(i + 1) * KC],
                        QT[pb:pb + 64, hp, j * QS:(j + 1) * QS],
                        start=True, stop=True)
                    E = sb.tile([KC, QS], F32R, name=f"E{s}", tag=f"E{s}",
                                bufs=3)
                    nc.scalar.activation(E[:], psS[:], AF.Exp)
                    if i in mtiles:
                        nc.vector.tensor_mul(E[:], E[:], mtiles[i][:])
                    pend.append((s, h, i, E))
                while len(pend) > 2:
                    s_, h_, i_, E_ = pend.pop(0)
                    nc.tensor.matmul(
                        psC[s_][:], Vaug[:, i_, h_, :], E_[:],
                        start=(i_ == first_i), stop=(i_ == last_i))
            for s_, h_, i_, E_ in pend:
                nc.tensor.matmul(
                    psC[s_][:], Vaug[:, i_, h_, :], E_[:],
                    start=(i_ == first_i), stop=(i_ == last_i))
            for s in range(2):
                h = 2 * hp + s
                rec = sb.tile([1, QS], F32R, name=f"rec{s}", tag=f"rec{s}",
                              bufs=2)
                with nc.allow_low_precision(reason="softmax denom recip"):
                    nc.vector.reciprocal(rec[:], psC[s][64:65, :])
                psB = ps.tile([64, QS], F32, name=f"psB{s}", tag=f"psS{s}",
                              bufs=2)
                nc.tensor.matmul(psB[:], ones_sb[0:1, 0:64], rec[:],
                                 start=True, stop=True)
                rb = sb.tile([64, QS], F32, name=f"rb{s}", tag=f"rb{s}",
                             bufs=2)
                nc.scalar.copy(rb[:], psB[:])
                ctx = sb.tile([64, QS], F32R, name=f"ctx{s}", tag=f"ctx{s}",
                              bufs=2)
                nc.vector.tensor_mul(ctx[:], psC[s][0:64, :], rb[:])
                nc.sync.dma_start(
                    cc_in[64 * h:64 * h + 64, j * QS:(j + 1) * QS], ctx[:])


def _qkvT_proj(nc, pools, xT, w_sb, b_sb, outT, pool_tag):
    """outT[:, pair, stripe] = w.T @ xT + b for 2 dout pairs x 4 stripes."""
    ps = pools['proj_ps']
    for pair in range(2):
        for jq in range(NQS):
            psq = ps.tile([128, QS], F32, name=f"psq_{pool_tag}",
                          tag=f"psq_{pool_tag}", bufs=2)
            for dc in range(NDC):
                nc.tensor.matmul(
                    psq[:],
                    w_sb[:, dc, pair * 128:(pair + 1) * 128],
                    xT[:, dc, jq * QS:(jq + 1) * QS],
                    start=(dc == 0), stop=(dc == NDC - 1))
            nc.scalar.activation(outT[:, pair, jq * QS:(jq + 1) * QS], psq[:],
                                 AF.Identity, bias=b_sb[:, pair:pair + 1])


def _v_proj(nc, pools, xT, wv_sb, bv_bc, Vaug, pool_tag):
    """Vaug[:, sb_i, h, 0:64] = (xT.T @ wv + bv) natural layout, 16 s-blocks."""
    ps = pools['proj_ps']
    for sb_i in range(NKC):
        psv = ps.tile([128, DH], F32, name=f"psv_{pool_tag}",
                      tag=f"psv_{pool_tag}", bufs=2)
        for dc in range(NDC):
            nc.tensor.matmul(
                psv[:],
                xT[:, dc, sb_i * KC:(sb_i + 1) * KC],
                wv_sb[:, dc, :],
                start=(dc == 0), stop=(dc == NDC - 1))
        nc.vector.tensor_tensor(
            out=Vaug[:, sb_i, :, 0:64],
            in0=psv[:].rearrange("p (h d) -> p h d", h=HPC),
            in1=bv_bc[:].rearrange("p (h d) -> p h d", h=HPC),
            op=OP.add)


def _layernorm_T(nc, pools, xraw, g_sb, b_sb, outT, ones_col, ones_sb, eps_sb,
                 pool_tag):
    """LayerNorm over d (partition-chunked) in transposed layout.
    xraw/outT: [128, 8, 512]. Stats via M=1 matmuls, broadcast via K=1."""
    sb = pools['ln_sb']
    ps = pools['ln_ps']
    pssum = ps.tile([1, Q4], F32, name=f"pssum_{pool_tag}", tag="pssum", bufs=1)
    pssq = ps.tile([1, Q4], F32, name=f"pssq_{pool_tag}", tag="pssq", bufs=1)
    for m in range(NDC):
        nc.tensor.matmul(pssum[:], ones_col[:, 0:1], xraw[:, m, :],
                         start=(m == 0), stop=(m == NDC - 1))
    for m in range(NDC):
        xsq = sb.tile([128, Q4], F32R, name="xsq", tag="xsq", bufs=2)
        nc.vector.tensor_mul(xsq[:], xraw[:, m, :], xraw[:, m, :])
        nc.tensor.matmul(pssq[:], ones_col[:, 0:1], xsq[:],
                         start=(m == 0), stop=(m == NDC - 1))
    mu = sb.tile([1, Q4], F32R, name="mu", tag="mu", bufs=2)
    nc.scalar.activation(mu[:], pssum[:], AF.Copy, scale=1.0 / D)
    msq = sb.tile([1, Q4], F32, name="msq", tag="msq", bufs=2)
    nc.scalar.activation(msq[:], pssq[:], AF.Copy, scale=1.0 / D)
    musq = sb.tile([1, Q4], F32, name="musq", tag="musq", bufs=2)
    nc.vector.tensor_mul(musq[:], mu[:], mu[:])
    var = sb.tile([1, Q4], F32, name="var", tag="var", bufs=2)
    nc.vector.tensor_tensor(out=var[:], in0=msq[:], in1=musq[:], op=OP.subtract)
    sd = sb.tile([1, Q4], F32, name="sd", tag="sd", bufs=2)
    nc.scalar.activation(sd[:], var[:], AF.Sqrt, bias=eps_sb[:])
    rstd = sb.tile([1, Q4], F32R, name="rstd", tag="rstd", bufs=2)
    with nc.allow_low_precision(reason="LN rstd recip"):
        nc.vector.reciprocal(rstd[:], sd[:])
    psmu = ps.tile([128, Q4], F32, name=f"psmu_{pool_tag}", tag="psmu", bufs=1)
    psrs = ps.tile([128, Q4], F32, name=f"psrs_{pool_tag}", tag="psrs", bufs=1)
    nc.tensor.matmul(psmu[:], ones_sb[0:1, :], mu[:], start=True, stop=True)
    nc.tensor.matmul(psrs[:], ones_sb[0:1, :], rstd[:], start=True, stop=True)
    mu_b = sb.tile([128, Q4], F32, name="mu_b", tag="mu_b", bufs=2)
    rs_b = sb.tile([128, Q4], F32, name="rs_b", tag="rs_b", bufs=2)
    nc.scalar.copy(mu_b[:], psmu[:])
    nc.scalar.copy(rs_b[:], psrs[:])
    for m in range(NDC):
        tmp = sb.tile([128, Q4], F32, name="lntmp", tag="lntmp", bufs=2)
        nc.vector.tensor_tensor(out=tmp[:], in0=xraw[:, m, :], in1=mu_b[:],
                                op=OP.subtract)
        nc.vector.tensor_mul(tmp[:], tmp[:], rs_b[:])
        nc.vector.tensor_scalar(
            out=outT[:, m, :], in0=tmp[:],
            scalar1=g_sb[:, m:m + 1], scalar2=b_sb[:, m:m + 1],
            op0=OP.mult, op1=OP.add)


def build(modes_sa, n_mask_sa, modes_ca, n_mask_ca, mask_index_sa,
          mask_index_ca):
    nc = bacc.Bacc("TRN2", num_devices=NCORES)

    # ---- DRAM I/O (all f32 bits; f32r where the PE consumes it) ----
    din = {}
    def dram_in(name, shape, dt=F32R):
        din[name] = nc.dram_tensor(name, shape, dt, kind="ExternalInput")
        return din[name]

    tgtT_d = dram_in("tgtT", [D, S])
    tgtTq_d = dram_in("tgtTq", [D, Q4], F32)
    memT_d = dram_in("memT", [D, S])
    wq_sa_d = dram_in("wq_sa", [D, DH])
    wk_sa_d = dram_in("wk_sa", [D, DH])
    wv_sa_d = dram_in("wv_sa", [D, DH])
    wo_sa_d = dram_in("wo_sa", [D, D])
    wq_ca_d = dram_in("wq_ca", [D, DH])
    wk_ca_d = dram_in("wk_ca", [D, DH])
    wv_ca_d = dram_in("wv_ca", [D, DH])
    wo_ca_d = dram_in("wo_ca", [D, D])
    w1_d = dram_in("w1", [D, DFF])
    w2_d = dram_in("w2", [DFF, D])
    bq_sa_d = dram_in("bq_sa", [DH], F32)
    bk_sa_d = dram_in("bk_sa", [DH], F32)
    bv_sa_d = dram_in("bv_sa", [DH], F32)
    bo_sa_d = dram_in("bo_sa", [D], F32)
    bq_ca_d = dram_in("bq_ca", [DH], F32)
    bk_ca_d = dram_in("bk_ca", [DH], F32)
    bv_ca_d = dram_in("bv_ca", [DH], F32)
    bo_ca_d = dram_in("bo_ca", [D], F32)
    b1_d = dram_in("b1", [DFF], F32)
    b2_d = dram_in("b2", [D], F32)
    ln_d = {}
    for i in (1, 2, 3):
        ln_d[f"g{i}"] = dram_in(f"ln{i}_g", [D], F32)
        ln_d[f"b{i}"] = dram_in(f"ln{i}_b", [D], F32)
    ones_d = dram_in("ones", [128, 128])
    mask_sa_d = dram_in("mask_sa", [max(n_mask_sa, 1), KC, QS])
    mask_ca_d = dram_in("mask_ca", [max(n_mask_ca, 1), KC, QS])
    out_d = nc.dram_tensor("out", [D, Q4], F32, kind="ExternalOutput")

    with tile.TileContext(nc) as tc:
        with (
            tc.tile_pool(name="persist", bufs=1) as persist,
            tc.tile_pool(name="dram", bufs=1, space="DRAM") as dram,
        ):
            # ---- collective scratch ----
            cc_in_sa = dram.tile([DH, S], F32R)
            cc_out_sa = dram.tile([NCORES * DH, S], F32R, addr_space="Shared")
            cc_in_x1 = dram.tile([D, Q4], F32R)
            cc_out_x1 = dram.tile([NCORES * D, Q4], F32R, addr_space="Shared")
            cc_in_ca = dram.tile([DH, S], F32R)
            cc_out_ca = dram.tile([NCORES * DH, S], F32R, addr_space="Shared")

            # runtime offsets from the SPMD partition id: my q-quarter within
            # the batch group, and my batch group's block in 8-rank AG outputs
            pid = nc.sync.partition_id()
            qoff = (pid % TP) * Q4
            ctx_boff = (pid // TP) * (TP * DH)
            x1_boff = (pid // TP) * (TP * D)

            # ---- small persistent constants ----
            ones_sb = persist.tile([1, 128], F32R)
            nc.sync.dma_start(ones_sb[:], ones_d[0:1, :])
            ones_col = persist.tile([128, 1], F32R)
            nc.sync.dma_start(ones_col[:], ones_d[:, 0:1])
            eps_sb = persist.tile([1, 1], F32)
            nc.vector.memset(eps_sb[:], EPS)

            def vec_sb(d, n):
                t = persist.tile([128, n // 128], F32,
                                 name=f"v_{d.name}", tag=f"v_{d.name}")
                nc.sync.dma_start(t[:], d[:].rearrange("(c p) -> p c", p=128))
                return t

            bq_sa_sb = vec_sb(bq_sa_d, DH)
            bk_sa_sb = vec_sb(bk_sa_d, DH)
            bo_sa_sb = vec_sb(bo_sa_d, D)
            bq_ca_sb = vec_sb(bq_ca_d, DH)
            bk_ca_sb = vec_sb(bk_ca_d, DH)
            bo_ca_sb = vec_sb(bo_ca_d, D)
            b1_sb = vec_sb(b1_d, DFF)
            b2_sb = vec_sb(b2_d, D)
            ln_sb = {k: vec_sb(v, D) for k, v in ln_d.items()}

            def bcast_sb(d, n):
                t = persist.tile([128, n], F32, name=f"bc_{d.name}",
                                 tag=f"bc_{d.name}")
                nc.gpsimd.dma_start(
                    out=t[:],
                    in_=bass.AP(tensor=d, offset=0, ap=[[0, 128], [1, n]]))
                return t

            bv_sa_bc = bcast_sb(bv_sa_d, DH)
            bv_ca_bc = bcast_sb(bv_ca_d, DH)

            pools = {}

            # ================= Phase 1: self-attn QKV =================
            with (
                tc.tile_pool(name="p1_w", bufs=1) as p1w,
                tc.tile_pool(name="p1_x", bufs=1) as p1x,
                tc.tile_pool(name="attn_sa_io", bufs=1) as sa_io,
            ):
                tgtT = p1x.tile([128, NDC, S], F32R)
                nc.sync.dma_start(
                    tgtT[:], tgtT_d[:].rearrange("(c p) s -> p c s", p=128))

                def load_w(d, cols):
                    t = p1w.tile([128, NDC, cols], F32R, name=f"w_{d.name}",
                                 tag=f"w_{d.name}")
                    nc.sync.dma_start(
                        t[:], d[:].rearrange("(c p) n -> p c n", p=128))
                    return t

                wq_sb = load_w(wq_sa_d, DH)
                wk_sb = load_w(wk_sa_d, DH)
                wv_sb = load_w(wv_sa_d, DH)

                QT_sa = sa_io.tile([128, 2, S], F32R)
                KT_sa = sa_io.tile([128, 2, S], F32R)
                Vaug_sa = sa_io.tile([128, NKC, HPC, 65], F32R)
                nc.sync.dma_start(
                    Vaug_sa[:, :, :, 64:65],
                    ones_d[:, 0:64].rearrange("p (a b c) -> p a b c",
                                              a=NKC, b=HPC))

                with tc.tile_pool(name="proj_ps", bufs=1,
                                  space="PSUM") as proj_ps:
                    pools['proj_ps'] = proj_ps
                    _qkvT_proj(nc, pools, tgtT, wq_sb, bq_sa_sb, QT_sa, "qsa")
                    _qkvT_proj(nc, pools, tgtT, wk_sb, bk_sa_sb, KT_sa, "ksa")
                    _v_proj(nc, pools, tgtT, wv_sb, bv_sa_bc, Vaug_sa, "vsa")

                # ============ Phase 2: self attention ============
                with (
                    tc.tile_pool(name="attn_sb", bufs=1) as attn_sb,
                    tc.tile_pool(name="attn_ps", bufs=1, space="PSUM") as attn_ps,
                ):
                    pools['attn_sb'] = attn_sb
                    pools['attn_ps'] = attn_ps
                    _attention(nc, tc, pools, QT_sa, KT_sa, Vaug_sa, modes_sa,
                               mask_index_sa, mask_sa_d, cc_in_sa, ones_sb)

            nc.gpsimd.collective_compute(
                "AllGather", OP.bypass, replica_groups=[list(range(NCORES))],
                ins=[cc_in_sa[:].opt()], outs=[cc_out_sa[:].opt()])

            # ===== Phase 3: cross-attn K/V (overlaps the AllToAll) =====
            with (
                tc.tile_pool(name="p3_w", bufs=1) as p3w,
                tc.tile_pool(name="attn_ca_io", bufs=1) as ca_io,
            ):
                def load_w3(d, cols):
                    t = p3w.tile([128, NDC, cols], F32R, name=f"w_{d.name}",
                                 tag=f"w_{d.name}")
                    nc.sync.dma_start(
                        t[:], d[:].rearrange("(c p) n -> p c n", p=128))
                    return t

                KT_ca = ca_io.tile([128, 2, S], F32R)
                Vaug_ca = ca_io.tile([128, NKC, HPC, 65], F32R)
                nc.sync.dma_start(
                    Vaug_ca[:, :, :, 64:65],
                    ones_d[:, 0:64].rearrange("p (a b c) -> p a b c",
                                              a=NKC, b=HPC))
                with (
                    tc.tile_pool(name="p3_x", bufs=1) as p3x,
                    tc.tile_pool(name="proj_ps2", bufs=1, space="PSUM") as proj_ps2,
                ):
                    pools['proj_ps'] = proj_ps2
                    memT = p3x.tile([128, NDC, S], F32R)
                    nc.sync.dma_start(
                        memT[:], memT_d[:].rearrange("(c p) s -> p c s", p=128))
                    wk_ca_sb = load_w3(wk_ca_d, DH)
                    wv_ca_sb = load_w3(wv_ca_d, DH)
                    _qkvT_proj(nc, pools, memT, wk_ca_sb, bk_ca_sb, KT_ca, "kca")
                    _v_proj(nc, pools, memT, wv_ca_sb, bv_ca_bc, Vaug_ca, "vca")

                # ====== Phase 4: self out-proj + residual + LN1 ======
                with (
                    tc.tile_pool(name="p4_sb", bufs=1) as p4sb,
                    tc.tile_pool(name="ln_sb", bufs=1) as ln_sb_pool,
                    tc.tile_pool(name="x1_keep", bufs=1) as x1_keep,
                    tc.tile_pool(name="p4_ps", bufs=1, space="PSUM") as p4ps,
                    tc.tile_pool(name="ln_ps", bufs=1, space="PSUM") as ln_ps,
                ):
                    pools['ln_sb'] = ln_sb_pool
                    pools['ln_ps'] = ln_ps
                    gctx = p4sb.tile([128, NDC, Q4], F32R)
                    for c in range(NDC):
                        nc.sync.dma_start(
                            gctx[:, c, :],
                            cc_out_sa[bass.ds(ctx_boff + 128 * c, 128),
                                      bass.ds(qoff, Q4)])
                    x1raw = p4sb.tile([128, NDC, Q4], F32R)
                    for m in range(NDC):
                        wom = p4sb.tile([128, NDC, 128], F32R, name="wom",
                                        tag="wom", bufs=2)
                        nc.sync.dma_start(
                            wom[:],
                            wo_sa_d[:, m * 128:(m + 1) * 128]
                            .rearrange("(c p) n -> p c n", p=128))
                        tqm = p4sb.tile([128, Q4], F32, name="tqm", tag="tqm",
                                        bufs=2)
                        nc.sync.dma_start(
                            tqm[:], tgtTq_d[m * 128:(m + 1) * 128, :])
                        pso = p4ps.tile([128, Q4], F32, name="pso", tag="pso",
                                        bufs=2)
                        for c in range(NDC):
                            nc.tensor.matmul(
                                pso[:], wom[:, c, :], gctx[:, c, :],
                                start=(c == 0), stop=(c == NDC - 1))
                        t_sb = p4sb.tile([128, Q4], F32, name="t_sb",
                                         tag="t_sb", bufs=2)
                        nc.scalar.activation(t_sb[:], pso[:], AF.Identity,
                                             bias=bo_sa_sb[:, m:m + 1])
                        nc.vector.tensor_tensor(out=x1raw[:, m, :], in0=t_sb[:],
                                                in1=tqm[:], op=OP.add)

                    x1T = x1_keep.tile([128, NDC, Q4], F32R)
                    _layernorm_T(nc, pools, x1raw, ln_sb["g1"], ln_sb["b1"],
                                 x1T, ones_col, ones_sb, eps_sb, "ln1")
                    nc.sync.dma_start(
                        cc_in_x1[:].rearrange("(c p) q -> p c q", p=128), x1T[:])

                nc.gpsimd.collective_compute(
                    "AllGather", OP.bypass, replica_groups=[list(range(NCORES))],
                    ins=[cc_in_x1[:].opt()], outs=[cc_out_x1[:].opt()])

                # ========= Phase 5: cross-attn Q projection =========
                QT_ca = ca_io.tile([128, 2, S], F32R)
                with (
                    tc.tile_pool(name="p5_sb", bufs=2) as p5sb,
                    tc.tile_pool(name="proj_ps3", bufs=1, space="PSUM") as proj_ps3,
                ):
                    wq_ca_sb = load_w3(wq_ca_d, DH)
                    for jq in range(NQS):
                        x1f = p5sb.tile([128, NDC, QS], F32R, name="x1f",
                                        tag="x1f")
                        nc.sync.dma_start(
                            x1f[:],
                            cc_out_x1[bass.ds(x1_boff + D * jq, D), :]
                            .rearrange("(c p) q -> p c q", p=128))
                        for pair in range(2):
                            psq = proj_ps3.tile([128, QS], F32, name="psq5",
                                                tag="psq5", bufs=2)
                            for dc in range(NDC):
                                nc.tensor.matmul(
                                    psq[:],
                                    wq_ca_sb[:, dc, pair * 128:(pair + 1) * 128],
                                    x1f[:, dc, :],
                                    start=(dc == 0), stop=(dc == NDC - 1))
                            nc.scalar.activation(
                                QT_ca[:, pair, jq * QS:(jq + 1) * QS], psq[:],
                                AF.Identity, bias=bq_ca_sb[:, pair:pair + 1])

                # ============ Phase 6: cross attention ============
                with (
                    tc.tile_pool(name="attn_sb2", bufs=1) as attn_sb2,
                    tc.tile_pool(name="attn_ps2", bufs=1, space="PSUM") as attn_ps2,
                ):
                    pools['attn_sb'] = attn_sb2
                    pools['attn_ps'] = attn_ps2
                    _attention(nc, tc, pools, QT_ca, KT_ca, Vaug_ca, modes_ca,
                               mask_index_ca, mask_ca_d, cc_in_ca, ones_sb)

            nc.gpsimd.collective_compute(
                "AllGather", OP.bypass, replica_groups=[list(range(NCORES))],
                ins=[cc_in_ca[:].opt()], outs=[cc_out_ca[:].opt()])

            # ====== Phase 7: cross out-proj + residual + LN2 ======
            x2T_pool = tc.alloc_tile_pool(name="x2_keep", bufs=1)
            with (
                tc.tile_pool(name="p7_sb", bufs=1) as p7sb,
                tc.tile_pool(name="ln_sb2", bufs=1) as ln_sb2,
                tc.tile_pool(name="p7_ps", bufs=1, space="PSUM") as p7ps,
                tc.tile_pool(name="ln_ps2", bufs=1, space="PSUM") as ln_ps2,
            ):
                pools['ln_sb'] = ln_sb2
                pools['ln_ps'] = ln_ps2
                gctx2 = p7sb.tile([128, NDC, Q4], F32R)
                for c in range(NDC):
                    nc.sync.dma_start(
                        gctx2[:, c, :],
                        cc_out_ca[bass.ds(ctx_boff + 128 * c, 128),
                                  bass.ds(qoff, Q4)])
                x2raw = p7sb.tile([128, NDC, Q4], F32R)
                for m in range(NDC):
                    wom2 = p7sb.tile([128, NDC, 128], F32R, name="wom2",
                                     tag="wom2", bufs=2)
                    nc.sync.dma_start(
                        wom2[:],
                        wo_ca_d[:, m * 128:(m + 1) * 128]
                        .rearrange("(c p) n -> p c n", p=128))
                    x1qm = p7sb.tile([128, Q4], F32R, name="x1qm", tag="x1qm",
                                     bufs=2)
                    nc.sync.dma_start(
                        x1qm[:], cc_in_x1[m * 128:(m + 1) * 128, :])
                    pso = p7ps.tile([128, Q4], F32, name="pso7", tag="pso7",
                                    bufs=2)
                    for c in range(NDC):
                        nc.tensor.matmul(
                            pso[:], wom2[:, c, :], gctx2[:, c, :],
                            start=(c == 0), stop=(c == NDC - 1))
                    t_sb = p7sb.tile([128, Q4], F32, name="t_sb7", tag="t_sb7",
                                     bufs=2)
                    nc.scalar.activation(t_sb[:], pso[:], AF.Identity,
                                         bias=bo_ca_sb[:, m:m + 1])
                    nc.vector.tensor_tensor(out=x2raw[:, m, :], in0=t_sb[:],
                                            in1=x1qm[:], op=OP.add)

                x2T = x2T_pool.tile([128, NDC, Q4], F32R)
                _layernorm_T(nc, pools, x2raw, ln_sb["g2"], ln_sb["b2"],
                             x2T, ones_col, ones_sb, eps_sb, "ln2")

            # ================= Phase 8: FFN + LN3 =================
            with (
                tc.tile_pool(name="p8_h", bufs=1) as p8h,
                tc.tile_pool(name="p8_w", bufs=3) as p8w,
                tc.tile_pool(name="p8_sb", bufs=1) as p8sb,
                tc.tile_pool(name="ln_sb3", bufs=1) as ln_sb3,
                tc.tile_pool(name="p8_ps", bufs=1, space="PSUM") as p8ps,
                tc.tile_pool(name="ln_ps3", bufs=1, space="PSUM") as ln_ps3,
            ):
                pools['ln_sb'] = ln_sb3
                pools['ln_ps'] = ln_ps3
                hT = p8h.tile([128, NFC, Q4], F32R)
                for f in range(NFC):
                    w1f = p8w.tile([128, NDC, 128], F32R, name="w1f", tag="w1f")
                    nc.sync.dma_start(
                        w1f[:],
                        w1_d[:, f * 128:(f + 1) * 128]
                        .rearrange("(c p) n -> p c n", p=128))
                    psh = p8ps.tile([128, Q4], F32, name="psh", tag="psh",
                                    bufs=2)
                    for m in range(NDC):
                        nc.tensor.matmul(psh[:], w1f[:, m, :], x2T[:, m, :],
                                         start=(m == 0), stop=(m == NDC - 1))
                    nc.scalar.activation(hT[:, f, :], psh[:], AF.Relu,
                                         bias=b1_sb[:, f:f + 1])
                x3raw = p8sb.tile([128, NDC, Q4], F32R)
                for m in range(NDC):
                    w2m = p8w.tile([128, NFC, 128], F32R, name="w2m", tag="w2m",
                                   bufs=2)
                    nc.sync.dma_start(
                        w2m[:],
                        w2_d[:, m * 128:(m + 1) * 128]
                        .rearrange("(c p) n -> p c n", p=128))
                    psf = p8ps.tile([128, Q4], F32, name="psf", tag="psf",
                                    bufs=2)
                    for f in range(NFC):
                        nc.tensor.matmul(psf[:], w2m[:, f, :], hT[:, f, :],
                                         start=(f == 0), stop=(f == NFC - 1))
                    t_sb = p8sb.tile([128, Q4], F32, name="t_sb8", tag="t_sb8",
                                     bufs=2)
                    nc.scalar.activation(t_sb[:], psf[:], AF.Identity,
                                         bias=b2_sb[:, m:m + 1])
                    nc.vector.tensor_tensor(out=x3raw[:, m, :], in0=t_sb[:],
                                            in1=x2T[:, m, :], op=OP.add)

                x3T = p8sb.tile([128, NDC, Q4], F32)
                _layernorm_T(nc, pools, x3raw, ln_sb["g3"], ln_sb["b3"],
                             x3T, ones_col, ones_sb, eps_sb, "ln3")
                nc.sync.dma_start(
                    out_d[:].rearrange("(c p) q -> p c q", p=128), x3T[:])
            x2T_pool.release()

    nc.finalize()
    return nc


_CACHE = {}


def _get_kernel(tgt_mask, memory_mask):
    modes_sa, tiles_sa, idx_sa = _analyze_mask(np.asarray(tgt_mask))
    modes_ca, tiles_ca, idx_ca = _analyze_mask(np.asarray(memory_mask))
    key = (tuple(map(tuple, modes_sa)), tuple(map(tuple, modes_ca)))
    if key not in _CACHE:
        nc = build(modes_sa, len(idx_sa), modes_ca, len(idx_ca), idx_sa, idx_ca)
        _CACHE[key] = nc
    return _CACHE[key], tiles_sa, tiles_ca


def _run(inputs, trace=False):
    tgt = np.asarray(inputs["tgt"], np.float32)
    memory = np.asarray(inputs["memory"], np.float32)
    nc, tiles_sa, tiles_ca = _get_kernel(inputs["tgt_mask"],
                                         inputs["memory_mask"])

    f32 = lambda x: np.ascontiguousarray(np.asarray(x), dtype=np.float32)
    ones128 = np.ones((128, 128), np.float32)
    shared = {
        "wo_sa": f32(inputs["sa_wo"]), "bo_sa": f32(inputs["sa_bo"]),
        "wo_ca": f32(inputs["ca_wo"]), "bo_ca": f32(inputs["ca_bo"]),
        "w1": f32(inputs["ff_w1"]), "b1": f32(inputs["ff_b1"]),
        "w2": f32(inputs["ff_w2"]), "b2": f32(inputs["ff_b2"]),
        "ln1_g": f32(inputs["ln1_g"]), "ln1_b": f32(inputs["ln1_b"]),
        "ln2_g": f32(inputs["ln2_g"]), "ln2_b": f32(inputs["ln2_b"]),
        "ln3_g": f32(inputs["ln3_g"]), "ln3_b": f32(inputs["ln3_b"]),
        "ones": ones128, "mask_sa": tiles_sa, "mask_ca": tiles_ca,
    }
    scale = 1.0 / np.sqrt(DK)
    in_maps = []
    for cid in range(NCORES):
        b, g = cid // TP, cid % TP
        hs = slice(g * DH, (g + 1) * DH)
        m = dict(shared)
        m["tgtT"] = f32(tgt[:, b, :].T)
        m["tgtTq"] = f32(tgt[g * Q4:(g + 1) * Q4, b, :].T)
        m["memT"] = f32(memory[:, b, :].T)
        m["wq_sa"] = f32(np.asarray(inputs["sa_wq"])[:, hs] * scale)
        m["bq_sa"] = f32(np.asarray(inputs["sa_bq"])[hs] * scale)
        m["wk_sa"] = f32(np.asarray(inputs["sa_wk"])[:, hs])
        m["bk_sa"] = f32(np.asarray(inputs["sa_bk"])[hs])
        m["wv_sa"] = f32(np.asarray(inputs["sa_wv"])[:, hs])
        m["bv_sa"] = f32(np.asarray(inputs["sa_bv"])[hs])
        m["wq_ca"] = f32(np.asarray(inputs["ca_wq"])[:, hs] * scale)
        m["bq_ca"] = f32(np.asarray(inputs["ca_bq"])[hs] * scale)
        m["wk_ca"] = f32(np.asarray(inputs["ca_wk"])[:, hs])
        m["bk_ca"] = f32(np.asarray(inputs["ca_bk"])[hs])
        m["wv_ca"] = f32(np.asarray(inputs["ca_wv"])[:, hs])
        m["bv_ca"] = f32(np.asarray(inputs["ca_bv"])[hs])
        in_maps.append(m)

    res = run_bass_kernel_spmd(nc, in_maps, core_ids=list(range(NCORES)),
                               trace=trace)
    out = np.empty((S, B, D), np.float32)
    for cid in range(NCORES):
        b, g = cid // TP, cid % TP
        out[g * Q4:(g + 1) * Q4, b, :] = res.results[cid]["out"].T
    return out, res


def kernel(**inputs):
    out, _ = _run(inputs, trace=False)
    return out


# revision 18
# speedup vs baseline: 1.3212x; 1.2233x over previous
"""Trainium2 Bass kernel for a transformer decoder block (self-attn + cross-attn + FFN).

Sharding: 8 cores = data-parallel over batch (2) x tensor-parallel over heads (4).
Attention QKV/scores/ctx are head-sharded; out-proj/LayerNorm/FFN are sharded over
query quarters. Cross-core resharding via AllToAll (ctx) and AllGather (x1).

Device layout: activations are kept transposed [d_model, seq] throughout. The host
pre-transposes inputs and re-transposes the output, so the device does zero
transposes. LayerNorm reductions over d (= partitions) are done with M=1 matmuls
against a ones column; broadcasts back over partitions with K=1 matmuls.
"""

import ml_dtypes
import numpy as np

import concourse.bass as bass
import concourse.mybir as mybir
import concourse.tile as tile
from concourse import bacc
from concourse.bass_utils import run_bass_kernel_spmd

F32 = mybir.dt.float32
F32R = mybir.dt.float32r
BF16 = mybir.dt.bfloat16
AF = mybir.ActivationFunctionType
OP = mybir.AluOpType

D = 1024
S = 2048
B = 2
NHEAD = 16
DK = 64
DFF = 4096
NCORES = 8
TP = 4            # tensor-parallel group size (heads)
HPC = NHEAD // TP  # heads per core = 4
DH = HPC * DK      # per-core head dim = 256
Q4 = S // TP       # query quarter = 512
QS = 512           # q stripe for attention
KC = 128           # k chunk
NQS = S // QS      # 4
NKC = S // KC      # 16
NDC = D // 128     # 8
NFC = DFF // 128   # 32
EPS = 1e-5

RG = [[0, 1, 2, 3], [4, 5, 6, 7]]


def _analyze_mask(mask):
    """Per (q-stripe j, k-chunk i) mode: 'free' (all 1), 'skip' (all 0), else
    'mask'. Also returns the masked tiles, transposed to [k, q], as an array
    [n_tiles, 128, 512] plus the (j, i) -> tile index map."""
    mt = mask.T  # [k, q]
    modes = [[None] * NKC for _ in range(NQS)]
    tiles = []
    index = {}
    for j in range(NQS):
        for i in range(NKC):
            sub = mt[i * KC:(i + 1) * KC, j * QS:(j + 1) * QS]
            if sub.all():
                modes[j][i] = 'free'
            elif not sub.any():
                modes[j][i] = 'skip'
            else:
                modes[j][i] = 'mask'
                index[(j, i)] = len(tiles)
                tiles.append(np.ascontiguousarray(sub, dtype=np.float32))
    tiles = np.stack(tiles) if tiles else np.zeros((1, KC, QS), np.float32)
    return modes, tiles, index


def _attention(nc, tc, pools, QT, KT, Vaug, modes, mask_index, mask_d, cc_in,
               ones_sb):
    """Head-TP attention in transposed layout. QT/KT: [128, 2, 2048] f32r
    (head h lives at partitions 64*(h%2).. of pchunk h//2). Vaug:
    [128, 16, 4, 65] f32r with ones in col 64. Writes normalized ctxT [64, 512]
    blocks to cc_in[j, 64h:64h+64, :]."""
    sb = pools['attn_sb']
    ps = pools['attn_ps']
    for j in range(NQS):
        mtiles = {}
        for i in range(NKC):
            if modes[j][i] == 'mask':
                mtile = sb.tile([KC, QS], BF16, name=f"mtile{len(mtiles) % 4}",
                                tag=f"mtile{len(mtiles) % 4}", bufs=2)
                nc.sync.dma_start(mtile[:], mask_d[mask_index[(j, i)], :, :])
                mtiles[i] = mtile
        live = [i for i in range(NKC) if modes[j][i] != 'skip']
        first_i, last_i = live[0], live[-1]
        for hp in range(2):
            psC = [ps.tile([65, QS], F32, name=f"psC{s}", tag=f"psC{s}", bufs=2)
                   for s in range(2)]
            # software-pipelined: ctx matmul for chunk i-1 is emitted after the
            # score matmuls for chunk i, so the in-order PE never waits on exp
            pend = []  # (s, h, i, E)
            for i in live:
                for s in range(2):
                    h = 2 * hp + s
                    pb = 64 * s
                    psS = ps.tile([KC, QS], F32, name=f"psS{s}", tag=f"psS{s}",
                                  bufs=2)
                    nc.tensor.matmul(
                        psS[:],
                        KT[pb:pb + 64, hp, i * KC:(i + 1) * KC],
                        QT[pb:pb + 64, hp, j * QS:(j + 1) * QS],
                        start=True, stop=True)
                    E = sb.tile([KC, QS], BF16, name=f"E{s}", tag=f"E{s}",
                                bufs=3)
                    nc.scalar.activation(E[:], psS[:], AF.Exp)
                    if i in mtiles:
                        nc.vector.tensor_mul(E[:], E[:], mtiles[i][:])
                    pend.append((s, h, i, E))
                while len(pend) > 2:
                    s_, h_, i_, E_ = pend.pop(0)
                    nc.tensor.matmul(
                        psC[s_][:], Vaug[:, i_, h_, :], E_[:],
                        start=(i_ == first_i), stop=(i_ == last_i))
            for s_, h_, i_, E_ in pend:
                nc.tensor.matmul(
                    psC[s_][:], Vaug[:, i_, h_, :], E_[:],
                    start=(i_ == first_i), stop=(i_ == last_i))
            for s in range(2):
                h = 2 * hp + s
                # evacuate psC to SBUF fast so the PE's next accumulation
                # group doesn't wait on the softmax-normalize chain
                cacc = sb.tile([65, QS], F32, name=f"cacc{s}", tag=f"cacc{s}",
                               bufs=2)
                nc.scalar.copy(cacc[:], psC[s][:])
                rec = sb.tile([1, QS], F32R, name=f"rec{s}", tag=f"rec{s}",
                              bufs=2)
                with nc.allow_low_precision(reason="softmax denom recip"):
                    nc.vector.reciprocal(rec[:], cacc[64:65, :])
                psB = ps.tile([64, QS], F32, name=f"psB{s}", tag=f"psS{s}",
                              bufs=2)
                nc.tensor.matmul(psB[:], ones_sb[0:1, 0:64], rec[:],
                                 start=True, stop=True)
                rb = sb.tile([64, QS], F32, name=f"rb{s}", tag=f"rb{s}",
                             bufs=2)
                nc.scalar.copy(rb[:], psB[:])
                ctx = sb.tile([64, QS], BF16, name=f"ctx{s}", tag=f"ctx{s}",
                              bufs=2)
                nc.vector.tensor_mul(ctx[:], cacc[0:64, :], rb[:])
                nc.sync.dma_start(
                    cc_in[64 * h:64 * h + 64, j * QS:(j + 1) * QS], ctx[:])


def _qkvT_proj(nc, pools, xT, w_sb, b_sb, outT, pool_tag):
    """outT[:, pair, stripe] = w.T @ xT + b for 2 dout pairs x 4 stripes."""
    ps = pools['proj_ps']
    for pair in range(2):
        for jq in range(NQS):
            psq = ps.tile([128, QS], F32, name=f"psq_{pool_tag}",
                          tag=f"psq_{pool_tag}", bufs=2)
            for dc in range(NDC):
                nc.tensor.matmul(
                    psq[:],
                    w_sb[:, dc, pair * 128:(pair + 1) * 128],
                    xT[:, dc, jq * QS:(jq + 1) * QS],
                    start=(dc == 0), stop=(dc == NDC - 1))
            nc.scalar.activation(outT[:, pair, jq * QS:(jq + 1) * QS], psq[:],
                                 AF.Identity, bias=b_sb[:, pair:pair + 1])


def _v_proj(nc, pools, xT, wv_sb, bv_bc, Vaug, pool_tag):
    """Vaug[:, sb_i, h, 0:64] = (xT.T @ wv + bv) natural layout, 16 s-blocks."""
    ps = pools['proj_ps']
    for sb_i in range(NKC):
        psv = ps.tile([128, DH], F32, name=f"psv_{pool_tag}",
                      tag=f"psv_{pool_tag}", bufs=2)
        for dc in range(NDC):
            nc.tensor.matmul(
                psv[:],
                xT[:, dc, sb_i * KC:(sb_i + 1) * KC],
                wv_sb[:, dc, :],
                start=(dc == 0), stop=(dc == NDC - 1))
        nc.vector.tensor_tensor(
            out=Vaug[:, sb_i, :, 0:64],
            in0=psv[:].rearrange("p (h d) -> p h d", h=HPC),
            in1=bv_bc[:].rearrange("p (h d) -> p h d", h=HPC),
            op=OP.add)


def _layernorm_T(nc, pools, xraw, g_sb, b_sb, outT, ones_col, ones_sb, eps_sb,
                 pool_tag, out_res=None):
    """LayerNorm over d (partition-chunked) in transposed layout.
    xraw/outT: [128, 8, 512]. Stats via M=1 matmuls, broadcast via K=1."""
    sb = pools['ln_sb']
    ps = pools['ln_ps']
    pssum = ps.tile([1, Q4], F32, name=f"pssum_{pool_tag}", tag="pssum", bufs=1)
    pssq = ps.tile([1, Q4], F32, name=f"pssq_{pool_tag}", tag="pssq", bufs=1)
    for m in range(NDC):
        nc.tensor.matmul(pssum[:], ones_col[:, 0:1], xraw[:, m, :],
                         start=(m == 0), stop=(m == NDC - 1))
    for m in range(NDC):
        xsq = sb.tile([128, Q4], F32R, name="xsq", tag="xsq", bufs=2)
        nc.vector.tensor_mul(xsq[:], xraw[:, m, :], xraw[:, m, :])
        nc.tensor.matmul(pssq[:], ones_col[:, 0:1], xsq[:],
                         start=(m == 0), stop=(m == NDC - 1))
    mu = sb.tile([1, Q4], F32R, name="mu", tag="mu", bufs=2)
    nc.scalar.activation(mu[:], pssum[:], AF.Copy, scale=1.0 / D)
    msq = sb.tile([1, Q4], F32, name="msq", tag="msq", bufs=2)
    nc.scalar.activation(msq[:], pssq[:], AF.Copy, scale=1.0 / D)
    musq = sb.tile([1, Q4], F32, name="musq", tag="musq", bufs=2)
    nc.vector.tensor_mul(musq[:], mu[:], mu[:])
    var = sb.tile([1, Q4], F32, name="var", tag="var", bufs=2)
    nc.vector.tensor_tensor(out=var[:], in0=msq[:], in1=musq[:], op=OP.subtract)
    sd = sb.tile([1, Q4], F32, name="sd", tag="sd", bufs=2)
    nc.scalar.activation(sd[:], var[:], AF.Sqrt, bias=eps_sb[:])
    rstd = sb.tile([1, Q4], F32R, name="rstd", tag="rstd", bufs=2)
    with nc.allow_low_precision(reason="LN rstd recip"):
        nc.vector.reciprocal(rstd[:], sd[:])
    psmu = ps.tile([128, Q4], F32, name=f"psmu_{pool_tag}", tag="psmu", bufs=1)
    psrs = ps.tile([128, Q4], F32, name=f"psrs_{pool_tag}", tag="psrs", bufs=1)
    nc.tensor.matmul(psmu[:], ones_sb[0:1, :], mu[:], start=True, stop=True)
    nc.tensor.matmul(psrs[:], ones_sb[0:1, :], rstd[:], start=True, stop=True)
    mu_b = sb.tile([128, Q4], F32, name="mu_b", tag="mu_b", bufs=2)
    rs_b = sb.tile([128, Q4], F32, name="rs_b", tag="rs_b", bufs=2)
    nc.scalar.copy(mu_b[:], psmu[:])
    nc.scalar.copy(rs_b[:], psrs[:])
    for m in range(NDC):
        tmp = sb.tile([128, Q4], F32, name="lntmp", tag="lntmp", bufs=2)
        nc.vector.tensor_tensor(out=tmp[:], in0=xraw[:, m, :], in1=mu_b[:],
                                op=OP.subtract)
        nc.vector.tensor_mul(tmp[:], tmp[:], rs_b[:])
        nc.vector.tensor_scalar(
            out=outT[:, m, :], in0=tmp[:],
            scalar1=g_sb[:, m:m + 1], scalar2=b_sb[:, m:m + 1],
            op0=OP.mult, op1=OP.add)
        if out_res is not None:
            nc.vector.tensor_scalar(
                out=out_res[:, m, :], in0=tmp[:],
                scalar1=g_sb[:, m:m + 1], scalar2=b_sb[:, m:m + 1],
                op0=OP.mult, op1=OP.add)


def build(modes_sa, n_mask_sa, modes_ca, n_mask_ca, mask_index_sa,
          mask_index_ca):
    nc = bacc.Bacc("TRN2", num_devices=NCORES)

    # ---- DRAM I/O (all f32 bits; f32r where the PE consumes it) ----
    din = {}
    def dram_in(name, shape, dt=F32R):
        din[name] = nc.dram_tensor(name, shape, dt, kind="ExternalInput")
        return din[name]

    tgtT_d = dram_in("tgtT", [D, S], BF16)
    tgtTq_d = dram_in("tgtTq", [D, Q4], F32)
    memT_d = dram_in("memT", [D, S], BF16)
    wq_sa_d = dram_in("wq_sa", [D, DH], BF16)
    wk_sa_d = dram_in("wk_sa", [D, DH], BF16)
    wv_sa_d = dram_in("wv_sa", [D, DH], BF16)
    wo_sa_d = dram_in("wo_sa", [D, D], BF16)
    wq_ca_d = dram_in("wq_ca", [D, DH], BF16)
    wk_ca_d = dram_in("wk_ca", [D, DH], BF16)
    wv_ca_d = dram_in("wv_ca", [D, DH], BF16)
    wo_ca_d = dram_in("wo_ca", [D, D], BF16)
    w1_d = dram_in("w1", [D, DFF], BF16)
    w2_d = dram_in("w2", [DFF, D], BF16)
    bq_sa_d = dram_in("bq_sa", [DH], F32)
    bk_sa_d = dram_in("bk_sa", [DH], F32)
    bv_sa_d = dram_in("bv_sa", [DH], F32)
    bo_sa_d = dram_in("bo_sa", [D], F32)
    bq_ca_d = dram_in("bq_ca", [DH], F32)
    bk_ca_d = dram_in("bk_ca", [DH], F32)
    bv_ca_d = dram_in("bv_ca", [DH], F32)
    bo_ca_d = dram_in("bo_ca", [D], F32)
    b1_d = dram_in("b1", [DFF], F32)
    b2_d = dram_in("b2", [D], F32)
    ln_d = {}
    for i in (1, 2, 3):
        ln_d[f"g{i}"] = dram_in(f"ln{i}_g", [D], F32)
        ln_d[f"b{i}"] = dram_in(f"ln{i}_b", [D], F32)
    ones_d = dram_in("ones", [128, 128])
    ones_h_d = dram_in("ones_h", [128, 128], BF16)
    mask_sa_d = dram_in("mask_sa", [max(n_mask_sa, 1), KC, QS], BF16)
    mask_ca_d = dram_in("mask_ca", [max(n_mask_ca, 1), KC, QS], BF16)
    out_d = nc.dram_tensor("out", [D, Q4], F32, kind="ExternalOutput")

    with tile.TileContext(nc) as tc:
        with (
            tc.tile_pool(name="persist", bufs=1) as persist,
            tc.tile_pool(name="dram", bufs=1, space="DRAM") as dram,
        ):
            # ---- collective scratch ----
            cc_in_sa = dram.tile([DH, S], BF16)
            cc_out_sa = dram.tile([NCORES * DH, S], BF16, addr_space="Shared")
            cc_in_x1 = dram.tile([D, Q4], BF16)
            cc_out_x1 = dram.tile([NCORES * D, Q4], BF16, addr_space="Shared")
            cc_in_ca = dram.tile([DH, S], BF16)
            cc_out_ca = dram.tile([NCORES * DH, S], BF16, addr_space="Shared")

            # runtime offsets from the SPMD partition id: my q-quarter within
            # the batch group, and my batch group's block in 8-rank AG outputs
            pid = nc.sync.partition_id()
            qoff = (pid % TP) * Q4
            ctx_boff = (pid // TP) * (TP * DH)
            x1_boff = (pid // TP) * (TP * D)

            # ---- small persistent constants ----
            ones_sb = persist.tile([1, 128], F32R)
            nc.sync.dma_start(ones_sb[:], ones_d[0:1, :])
            ones_col = persist.tile([128, 1], F32R)
            nc.sync.dma_start(ones_col[:], ones_d[:, 0:1])
            eps_sb = persist.tile([1, 1], F32)
            nc.vector.memset(eps_sb[:], EPS)

            def vec_sb(d, n):
                t = persist.tile([128, n // 128], F32,
                                 name=f"v_{d.name}", tag=f"v_{d.name}")
                nc.sync.dma_start(t[:], d[:].rearrange("(c p) -> p c", p=128))
                return t

            bq_sa_sb = vec_sb(bq_sa_d, DH)
            bk_sa_sb = vec_sb(bk_sa_d, DH)
            bo_sa_sb = vec_sb(bo_sa_d, D)
            bq_ca_sb = vec_sb(bq_ca_d, DH)
            bk_ca_sb = vec_sb(bk_ca_d, DH)
            bo_ca_sb = vec_sb(bo_ca_d, D)
            b1_sb = vec_sb(b1_d, DFF)
            b2_sb = vec_sb(b2_d, D)
            ln_sb = {k: vec_sb(v, D) for k, v in ln_d.items()}

            def bcast_sb(d, n):
                t = persist.tile([128, n], F32, name=f"bc_{d.name}",
                                 tag=f"bc_{d.name}")
                nc.gpsimd.dma_start(
                    out=t[:],
                    in_=bass.AP(tensor=d, offset=0, ap=[[0, 128], [1, n]]))
                return t

            bv_sa_bc = bcast_sb(bv_sa_d, DH)
            bv_ca_bc = bcast_sb(bv_ca_d, DH)

            pools = {}

            x1res_pool = tc.alloc_tile_pool(name="x1res_keep", bufs=1)
            x2res_pool = tc.alloc_tile_pool(name="x2res_keep", bufs=1)

            # ================= Phase 1: self-attn QKV =================
            with (
                tc.tile_pool(name="p1_w", bufs=1) as p1w,
                tc.tile_pool(name="p1_x", bufs=1) as p1x,
                tc.tile_pool(name="attn_sa_io", bufs=1) as sa_io,
            ):
                tgtT = p1x.tile([128, NDC, S], BF16)
                nc.sync.dma_start(
                    tgtT[:], tgtT_d[:].rearrange("(c p) s -> p c s", p=128))

                def load_w(d, cols):
                    t = p1w.tile([128, NDC, cols], BF16, name=f"w_{d.name}",
                                 tag=f"w_{d.name}")
                    nc.sync.dma_start(
                        t[:], d[:].rearrange("(c p) n -> p c n", p=128))
                    return t

                wq_sb = load_w(wq_sa_d, DH)
                wk_sb = load_w(wk_sa_d, DH)
                wv_sb = load_w(wv_sa_d, DH)

                QT_sa = sa_io.tile([128, 2, S], BF16)
                KT_sa = sa_io.tile([128, 2, S], BF16)
                Vaug_sa = sa_io.tile([128, NKC, HPC, 65], BF16)
                nc.sync.dma_start(
                    Vaug_sa[:, :, :, 64:65],
                    ones_h_d[:, 0:64].rearrange("p (a b c) -> p a b c",
                                              a=NKC, b=HPC))

                with tc.tile_pool(name="proj_ps", bufs=1,
                                  space="PSUM") as proj_ps:
                    pools['proj_ps'] = proj_ps
                    _qkvT_proj(nc, pools, tgtT, wq_sb, bq_sa_sb, QT_sa, "qsa")
                    _qkvT_proj(nc, pools, tgtT, wk_sb, bk_sa_sb, KT_sa, "ksa")
                    _v_proj(nc, pools, tgtT, wv_sb, bv_sa_bc, Vaug_sa, "vsa")

                # ============ Phase 2: self attention ============
                with (
                    tc.tile_pool(name="attn_sb", bufs=1) as attn_sb,
                    tc.tile_pool(name="attn_ps", bufs=1, space="PSUM") as attn_ps,
                ):
                    pools['attn_sb'] = attn_sb
                    pools['attn_ps'] = attn_ps
                    _attention(nc, tc, pools, QT_sa, KT_sa, Vaug_sa, modes_sa,
                               mask_index_sa, mask_sa_d, cc_in_sa, ones_sb)

            nc.gpsimd.collective_compute(
                "AllGather", OP.bypass, replica_groups=[list(range(NCORES))],
                ins=[cc_in_sa[:].opt()], outs=[cc_out_sa[:].opt()])

            # ===== Phase 3: cross-attn K/V (overlaps the AllToAll) =====
            with (
                tc.tile_pool(name="p3_w", bufs=1) as p3w,
                tc.tile_pool(name="attn_ca_io", bufs=1) as ca_io,
            ):
                def load_w3(d, cols):
                    t = p3w.tile([128, NDC, cols], BF16, name=f"w_{d.name}",
                                 tag=f"w_{d.name}")
                    nc.sync.dma_start(
                        t[:], d[:].rearrange("(c p) n -> p c n", p=128))
                    return t

                KT_ca = ca_io.tile([128, 2, S], BF16)
                Vaug_ca = ca_io.tile([128, NKC, HPC, 65], BF16)
                nc.sync.dma_start(
                    Vaug_ca[:, :, :, 64:65],
                    ones_h_d[:, 0:64].rearrange("p (a b c) -> p a b c",
                                              a=NKC, b=HPC))
                with (
                    tc.tile_pool(name="p3_x", bufs=1) as p3x,
                    tc.tile_pool(name="proj_ps2", bufs=1, space="PSUM") as proj_ps2,
                ):
                    pools['proj_ps'] = proj_ps2
                    memT = p3x.tile([128, NDC, S], BF16)
                    nc.sync.dma_start(
                        memT[:], memT_d[:].rearrange("(c p) s -> p c s", p=128))
                    wk_ca_sb = load_w3(wk_ca_d, DH)
                    wv_ca_sb = load_w3(wv_ca_d, DH)
                    _qkvT_proj(nc, pools, memT, wk_ca_sb, bk_ca_sb, KT_ca, "kca")
                    _v_proj(nc, pools, memT, wv_ca_sb, bv_ca_bc, Vaug_ca, "vca")

                # ====== Phase 4: self out-proj + residual + LN1 ======
                with (
                    tc.tile_pool(name="p4_sb", bufs=1) as p4sb,
                    tc.tile_pool(name="ln_sb", bufs=1) as ln_sb_pool,
                    tc.tile_pool(name="x1_keep", bufs=1) as x1_keep,
                    tc.tile_pool(name="p4_ps", bufs=1, space="PSUM") as p4ps,
                    tc.tile_pool(name="ln_ps", bufs=1, space="PSUM") as ln_ps,
                ):
                    pools['ln_sb'] = ln_sb_pool
                    pools['ln_ps'] = ln_ps
                    gctx = p4sb.tile([128, NDC, Q4], BF16)
                    for c in range(NDC):
                        nc.sync.dma_start(
                            gctx[:, c, :],
                            cc_out_sa[bass.ds(ctx_boff + 128 * c, 128),
                                      bass.ds(qoff, Q4)])
                    x1raw = p4sb.tile([128, NDC, Q4], F32R)
                    for m in range(NDC):
                        wom = p4sb.tile([128, NDC, 128], BF16, name="wom",
                                        tag="wom", bufs=2)
                        nc.sync.dma_start(
                            wom[:],
                            wo_sa_d[:, m * 128:(m + 1) * 128]
                            .rearrange("(c p) n -> p c n", p=128))
                        tqm = p4sb.tile([128, Q4], F32, name="tqm", tag="tqm",
                                        bufs=2)
                        nc.sync.dma_start(
                            tqm[:], tgtTq_d[m * 128:(m + 1) * 128, :])
                        pso = p4ps.tile([128, Q4], F32, name="pso", tag="pso",
                                        bufs=2)
                        for c in range(NDC):
                            nc.tensor.matmul(
                                pso[:], wom[:, c, :], gctx[:, c, :],
                                start=(c == 0), stop=(c == NDC - 1))
                        t_sb = p4sb.tile([128, Q4], F32, name="t_sb",
                                         tag="t_sb", bufs=2)
                        nc.scalar.activation(t_sb[:], pso[:], AF.Identity,
                                             bias=bo_sa_sb[:, m:m + 1])
                        nc.vector.tensor_tensor(out=x1raw[:, m, :], in0=t_sb[:],
                                                in1=tqm[:], op=OP.add)

                    x1T = x1_keep.tile([128, NDC, Q4], BF16)
                    x1res = x1res_pool.tile([128, NDC, Q4], F32)
                    _layernorm_T(nc, pools, x1raw, ln_sb["g1"], ln_sb["b1"],
                                 x1T, ones_col, ones_sb, eps_sb, "ln1",
                                 out_res=x1res)
                    nc.sync.dma_start(
                        cc_in_x1[:].rearrange("(c p) q -> p c q", p=128), x1T[:])

                nc.gpsimd.collective_compute(
                    "AllGather", OP.bypass, replica_groups=[list(range(NCORES))],
                    ins=[cc_in_x1[:].opt()], outs=[cc_out_x1[:].opt()])

                # ========= Phase 5: cross-attn Q projection =========
                QT_ca = ca_io.tile([128, 2, S], BF16)
                with (
                    tc.tile_pool(name="p5_sb", bufs=2) as p5sb,
                    tc.tile_pool(name="proj_ps3", bufs=1, space="PSUM") as proj_ps3,
                ):
                    wq_ca_sb = load_w3(wq_ca_d, DH)
                    for jq in range(NQS):
                        x1f = p5sb.tile([128, NDC, QS], BF16, name="x1f",
                                        tag="x1f")
                        nc.sync.dma_start(
                            x1f[:],
                            cc_out_x1[bass.ds(x1_boff + D * jq, D), :]
                            .rearrange("(c p) q -> p c q", p=128))
                        for pair in range(2):
                            psq = proj_ps3.tile([128, QS], F32, name="psq5",
                                                tag="psq5", bufs=2)
                            for dc in range(NDC):
                                nc.tensor.matmul(
                                    psq[:],
                                    wq_ca_sb[:, dc, pair * 128:(pair + 1) * 128],
                                    x1f[:, dc, :],
                                    start=(dc == 0), stop=(dc == NDC - 1))
                            nc.scalar.activation(
                                QT_ca[:, pair, jq * QS:(jq + 1) * QS], psq[:],
                                AF.Identity, bias=bq_ca_sb[:, pair:pair + 1])

                # ============ Phase 6: cross attention ============
                with (
                    tc.tile_pool(name="attn_sb2", bufs=1) as attn_sb2,
                    tc.tile_pool(name="attn_ps2", bufs=1, space="PSUM") as attn_ps2,
                ):
                    pools['attn_sb'] = attn_sb2
                    pools['attn_ps'] = attn_ps2
                    _attention(nc, tc, pools, QT_ca, KT_ca, Vaug_ca, modes_ca,
                               mask_index_ca, mask_ca_d, cc_in_ca, ones_sb)

            nc.gpsimd.collective_compute(
                "AllGather", OP.bypass, replica_groups=[list(range(NCORES))],
                ins=[cc_in_ca[:].opt()], outs=[cc_out_ca[:].opt()])

            # ====== Phase 7: cross out-proj + residual + LN2 ======
            x2T_pool = tc.alloc_tile_pool(name="x2_keep", bufs=1)
            with (
                tc.tile_pool(name="p7_sb", bufs=1) as p7sb,
                tc.tile_pool(name="ln_sb2", bufs=1) as ln_sb2,
                tc.tile_pool(name="p7_ps", bufs=1, space="PSUM") as p7ps,
                tc.tile_pool(name="ln_ps2", bufs=1, space="PSUM") as ln_ps2,
            ):
                pools['ln_sb'] = ln_sb2
                pools['ln_ps'] = ln_ps2
                gctx2 = p7sb.tile([128, NDC, Q4], BF16)
                for c in range(NDC):
                    nc.sync.dma_start(
                        gctx2[:, c, :],
                        cc_out_ca[bass.ds(ctx_boff + 128 * c, 128),
                                  bass.ds(qoff, Q4)])
                x2raw = p7sb.tile([128, NDC, Q4], F32R)
                for m in range(NDC):
                    wom2 = p7sb.tile([128, NDC, 128], BF16, name="wom2",
                                     tag="wom2", bufs=2)
                    nc.sync.dma_start(
                        wom2[:],
                        wo_ca_d[:, m * 128:(m + 1) * 128]
                        .rearrange("(c p) n -> p c n", p=128))
                    pso = p7ps.tile([128, Q4], F32, name="pso7", tag="pso7",
                                    bufs=2)
                    for c in range(NDC):
                        nc.tensor.matmul(
                            pso[:], wom2[:, c, :], gctx2[:, c, :],
                            start=(c == 0), stop=(c == NDC - 1))
                    t_sb = p7sb.tile([128, Q4], F32, name="t_sb7", tag="t_sb7",
                                     bufs=2)
                    nc.scalar.activation(t_sb[:], pso[:], AF.Identity,
                                         bias=bo_ca_sb[:, m:m + 1])
                    nc.vector.tensor_tensor(out=x2raw[:, m, :], in0=t_sb[:],
                                            in1=x1res[:, m, :], op=OP.add)

                x2T = x2T_pool.tile([128, NDC, Q4], BF16)
                x2res = x2res_pool.tile([128, NDC, Q4], F32)
                _layernorm_T(nc, pools, x2raw, ln_sb["g2"], ln_sb["b2"],
                             x2T, ones_col, ones_sb, eps_sb, "ln2",
                             out_res=x2res)

            # ================= Phase 8: FFN + LN3 =================
            with (
                tc.tile_pool(name="p8_h", bufs=1) as p8h,
                tc.tile_pool(name="p8_w", bufs=3) as p8w,
                tc.tile_pool(name="p8_sb", bufs=1) as p8sb,
                tc.tile_pool(name="ln_sb3", bufs=1) as ln_sb3,
                tc.tile_pool(name="p8_ps", bufs=1, space="PSUM") as p8ps,
                tc.tile_pool(name="ln_ps3", bufs=1, space="PSUM") as ln_ps3,
            ):
                pools['ln_sb'] = ln_sb3
                pools['ln_ps'] = ln_ps3
                hT = p8h.tile([128, NFC, Q4], BF16)
                for f in range(NFC):
                    w1f = p8w.tile([128, NDC, 128], BF16, name="w1f", tag="w1f")
                    nc.sync.dma_start(
                        w1f[:],
                        w1_d[:, f * 128:(f + 1) * 128]
                        .rearrange("(c p) n -> p c n", p=128))
                    psh = p8ps.tile([128, Q4], F32, name="psh", tag="psh",
                                    bufs=2)
                    for m in range(NDC):
                        nc.tensor.matmul(psh[:], w1f[:, m, :], x2T[:, m, :],
                                         start=(m == 0), stop=(m == NDC - 1))
                    nc.scalar.activation(hT[:, f, :], psh[:], AF.Relu,
                                         bias=b1_sb[:, f:f + 1])
                x3raw = p8sb.tile([128, NDC, Q4], F32R)
                for m in range(NDC):
                    w2m = p8w.tile([128, NFC, 128], BF16, name="w2m", tag="w2m",
                                   bufs=2)
                    nc.sync.dma_start(
                        w2m[:],
                        w2_d[:, m * 128:(m + 1) * 128]
                        .rearrange("(c p) n -> p c n", p=128))
                    psf = p8ps.tile([128, Q4], F32, name="psf", tag="psf",
                                    bufs=2)
                    for f in range(NFC):
                        nc.tensor.matmul(psf[:], w2m[:, f, :], hT[:, f, :],
                                         start=(f == 0), stop=(f == NFC - 1))
                    t_sb = p8sb.tile([128, Q4], F32, name="t_sb8", tag="t_sb8",
                                     bufs=2)
                    nc.scalar.activation(t_sb[:], psf[:], AF.Identity,
                                         bias=b2_sb[:, m:m + 1])
                    nc.vector.tensor_tensor(out=x3raw[:, m, :], in0=t_sb[:],
                                            in1=x2res[:, m, :], op=OP.add)

                x3T = p8sb.tile([128, NDC, Q4], F32)
                _layernorm_T(nc, pools, x3raw, ln_sb["g3"], ln_sb["b3"],
                             x3T, ones_col, ones_sb, eps_sb, "ln3")
                nc.sync.dma_start(
                    out_d[:].rearrange("(c p) q -> p c q", p=128), x3T[:])
            x2T_pool.release()
            x2res_pool.release()
            x1res_pool.release()

    nc.finalize()
    return nc


_CACHE = {}


def _get_kernel(tgt_mask, memory_mask):
    modes_sa, tiles_sa, idx_sa = _analyze_mask(np.asarray(tgt_mask))
    modes_ca, tiles_ca, idx_ca = _analyze_mask(np.asarray(memory_mask))
    key = (tuple(map(tuple, modes_sa)), tuple(map(tuple, modes_ca)))
    if key not in _CACHE:
        nc = build(modes_sa, len(idx_sa), modes_ca, len(idx_ca), idx_sa, idx_ca)
        _CACHE[key] = nc
    return _CACHE[key], tiles_sa, tiles_ca


def _run(inputs, trace=False):
    tgt = np.asarray(inputs["tgt"], np.float32)
    memory = np.asarray(inputs["memory"], np.float32)
    nc, tiles_sa, tiles_ca = _get_kernel(inputs["tgt_mask"],
                                         inputs["memory_mask"])

    f32 = lambda x: np.ascontiguousarray(np.asarray(x), dtype=np.float32)
    bf = lambda x: np.ascontiguousarray(
        np.asarray(x, dtype=np.float32).astype(ml_dtypes.bfloat16))
    ones128 = np.ones((128, 128), np.float32)
    shared = {
        "wo_sa": bf(inputs["sa_wo"]), "bo_sa": f32(inputs["sa_bo"]),
        "wo_ca": bf(inputs["ca_wo"]), "bo_ca": f32(inputs["ca_bo"]),
        "w1": bf(inputs["ff_w1"]), "b1": f32(inputs["ff_b1"]),
        "w2": bf(inputs["ff_w2"]), "b2": f32(inputs["ff_b2"]),
        "ln1_g": f32(inputs["ln1_g"]), "ln1_b": f32(inputs["ln1_b"]),
        "ln2_g": f32(inputs["ln2_g"]), "ln2_b": f32(inputs["ln2_b"]),
        "ln3_g": f32(inputs["ln3_g"]), "ln3_b": f32(inputs["ln3_b"]),
        "ones": ones128, "ones_h": ones128.astype(ml_dtypes.bfloat16),
        "mask_sa": bf(tiles_sa), "mask_ca": bf(tiles_ca),
    }
    scale = 1.0 / np.sqrt(DK)
    in_maps = []
    for cid in range(NCORES):
        b, g = cid // TP, cid % TP
        hs = slice(g * DH, (g + 1) * DH)
        m = dict(shared)
        m["tgtT"] = bf(tgt[:, b, :].T)
        m["tgtTq"] = f32(tgt[g * Q4:(g + 1) * Q4, b, :].T)
        m["memT"] = bf(memory[:, b, :].T)
        m["wq_sa"] = bf(np.asarray(inputs["sa_wq"])[:, hs] * scale)
        m["bq_sa"] = f32(np.asarray(inputs["sa_bq"])[hs] * scale)
        m["wk_sa"] = bf(np.asarray(inputs["sa_wk"])[:, hs])
        m["bk_sa"] = f32(np.asarray(inputs["sa_bk"])[hs])
        m["wv_sa"] = bf(np.asarray(inputs["sa_wv"])[:, hs])
        m["bv_sa"] = f32(np.asarray(inputs["sa_bv"])[hs])
        m["wq_ca"] = bf(np.asarray(inputs["ca_wq"])[:, hs] * scale)
        m["bq_ca"] = f32(np.asarray(inputs["ca_bq"])[hs] * scale)
        m["wk_ca"] = bf(np.asarray(inputs["ca_wk"])[:, hs])
        m["bk_ca"] = f32(np.asarray(inputs["ca_bk"])[hs])
        m["wv_ca"] = bf(np.asarray(inputs["ca_wv"])[:, hs])
        m["bv_ca"] = f32(np.asarray(inputs["ca_bv"])[hs])
        in_maps.append(m)

    res = run_bass_kernel_spmd(nc, in_maps, core_ids=list(range(NCORES)),
                               trace=trace)
    out = np.empty((S, B, D), np.float32)
    for cid in range(NCORES):
        b, g = cid // TP, cid % TP
        out[g * Q4:(g + 1) * Q4, b, :] = res.results[cid]["out"].T
    return out, res


def kernel(**inputs):
    out, _ = _run(inputs, trace=False)
    return out


# revision 21
# speedup vs baseline: 1.5275x; 1.1561x over previous
"""Trainium2 Bass kernel for a transformer decoder block (self-attn + cross-attn + FFN).

Sharding: 8 cores = data-parallel over batch (2) x tensor-parallel over heads (4).
Attention QKV/scores/ctx are head-sharded; out-proj/LayerNorm/FFN are sharded over
query quarters (quarter g == attention stripe g). Cross-core resharding via
8-rank AllGathers: per-stripe for attention context (overlapped with compute),
one for x1.

Device layout: activations are transposed [d_model, seq] throughout; the host
pre-transposes inputs and re-transposes the output, so the device does zero
transposes. LayerNorm reductions over d (= partitions) use M=1 matmuls against a
ones column; partition-broadcasts use K=1 matmuls. Softmax skips the max
subtraction (scores are bounded by construction) and folds the denominator into
an extra all-ones column of V; masking is multiplicative post-exp, with
host-analyzed per-chunk modes (free / masked-tile / skip) so causal masks skip
half the work structurally.

Matmuls run in bf16 (except tiny LN/broadcast ones in f32r); the residual / LN
statistics backbone stays in f32/f32r.
"""

import ml_dtypes
import numpy as np

import concourse.bass as bass
import concourse.mybir as mybir
import concourse.tile as tile
from concourse import bacc
from concourse.bass_utils import run_bass_kernel_spmd

F32 = mybir.dt.float32
F32R = mybir.dt.float32r
BF16 = mybir.dt.bfloat16
AF = mybir.ActivationFunctionType
OP = mybir.AluOpType

D = 1024
S = 2048
B = 2
NHEAD = 16
DK = 64
DFF = 4096
NCORES = 8
TP = 4             # tensor-parallel group size (heads)
HPC = NHEAD // TP  # heads per core = 4
DH = HPC * DK      # per-core head dim = 256
Q4 = S // TP       # query quarter = 512 (== attention q stripe)
QS = 512           # q stripe for attention
KC = 128           # k chunk
NQS = S // QS      # 4
NKC = S // KC      # 16
NDC = D // 128     # 8
NFC = DFF // 128   # 32
EPS = 1e-5
ALLR = [list(range(NCORES))]


def _analyze_mask(mask):
    """Per (q-stripe j, k-chunk i) mode: 'free' (all 1), 'skip' (all 0), else
    'mask'. Masked tiles are returned transposed to [k, q] as
    [n_tiles, 128, 512] plus a (j, i) -> tile index map."""
    mt = mask.T  # [k, q]
    modes = [[None] * NKC for _ in range(NQS)]
    tiles = []
    index = {}
    for j in range(NQS):
        for i in range(NKC):
            sub = mt[i * KC:(i + 1) * KC, j * QS:(j + 1) * QS]
            if sub.all():
                modes[j][i] = 'free'
            elif not sub.any():
                modes[j][i] = 'skip'
            else:
                modes[j][i] = 'mask'
                index[(j, i)] = len(tiles)
                tiles.append(np.ascontiguousarray(sub, dtype=np.float32))
    tiles = np.stack(tiles) if tiles else np.zeros((1, KC, QS), np.float32)
    return modes, tiles, index


def _attention(nc, tc, sb, ps, QT, KT, Vaug, modes, mask_index, mask_d, cc_in,
               cc_out, ones_sb, psc_bufs, after_stripe=None):
    """Head-TP attention in transposed layout. QT/KT: [128, 2, 2048] bf16
    (head h at partitions 64*(h%2).. of pchunk h//2). Vaug: [128, 16, 4, 65]
    bf16 with ones in col 64. Per stripe j: writes normalized ctxT blocks to
    cc_in[j], AllGathers cc_in[j] into cc_out's stripe block, then calls
    after_stripe(j) to emit filler work overlapping the collective."""
    for j in range(NQS):
        mtiles = {}
        for i in range(NKC):
            if modes[j][i] == 'mask':
                mtile = sb.tile([KC, QS], BF16, name=f"mtile{len(mtiles) % 4}",
                                tag=f"mtile{len(mtiles) % 4}", bufs=2)
                nc.sync.dma_start(mtile[:], mask_d[mask_index[(j, i)], :, :])
                mtiles[i] = mtile
        live = [i for i in range(NKC) if modes[j][i] != 'skip']
        first_i, last_i = live[0], live[-1]
        for hp in range(2):
            psC = [ps.tile([65, QS], F32, name=f"psC{s}", tag=f"psC{s}",
                           bufs=psc_bufs) for s in range(2)]
            pend = []  # (i, E2) with both heads' exp in one 2-bank tile
            for i in live:
                psS = ps.tile([KC, 2, QS], F32, name="psS", tag="psS", bufs=2)
                for s in range(2):
                    pb = 64 * s
                    nc.tensor.matmul(
                        psS[:, s, :],
                        KT[pb:pb + 64, hp, i * KC:(i + 1) * KC],
                        QT[pb:pb + 64, hp, j * QS:(j + 1) * QS],
                        start=True, stop=True)
                E2 = sb.tile([KC, 2, QS], BF16, name="E2", tag="E2", bufs=3)
                nc.scalar.activation(E2[:], psS[:], AF.Exp)
                if i in mtiles:
                    for s in range(2):
                        nc.vector.tensor_mul(E2[:, s, :], E2[:, s, :],
                                             mtiles[i][:])
                pend.append((i, E2))
                if len(pend) > 1:
                    i_, E_ = pend.pop(0)
                    for s in range(2):
                        nc.tensor.matmul(
                            psC[s][:], Vaug[:, i_, 2 * hp + s, :], E_[:, s, :],
                            start=(i_ == first_i), stop=(i_ == last_i))
            for i_, E_ in pend:
                for s in range(2):
                    nc.tensor.matmul(
                        psC[s][:], Vaug[:, i_, 2 * hp + s, :], E_[:, s, :],
                        start=(i_ == first_i), stop=(i_ == last_i))
            for s in range(2):
                h = 2 * hp + s
                # evacuate psC fast so the PE's next accumulation group
                # doesn't wait on the softmax-normalize chain
                cacc = sb.tile([65, QS], F32, name=f"cacc{s}", tag=f"cacc{s}",
                               bufs=2)
                nc.scalar.copy(cacc[:], psC[s][:])
                rec = sb.tile([1, QS], F32R, name=f"rec{s}", tag=f"rec{s}",
                              bufs=2)
                with nc.allow_low_precision(reason="softmax denom recip"):
                    nc.vector.reciprocal(rec[:], cacc[64:65, :])
                psB = ps.tile([64, QS], F32, name=f"psB{s}", tag="psS", bufs=2)
                nc.tensor.matmul(psB[:], ones_sb[0:1, 0:64], rec[:],
                                 start=True, stop=True)
                rb = sb.tile([64, QS], F32, name=f"rb{s}", tag=f"rb{s}",
                             bufs=2)
                nc.scalar.copy(rb[:], psB[:])
                ctx = sb.tile([64, QS], BF16, name=f"ctx{s}", tag=f"ctx{s}",
                              bufs=2)
                nc.vector.tensor_mul(ctx[:], cacc[0:64, :], rb[:])
                nc.sync.dma_start(cc_in[j, 64 * h:64 * h + 64, :], ctx[:])
        nc.gpsimd.collective_compute(
            "AllGather", OP.bypass, replica_groups=ALLR,
            ins=[cc_in[j, :, :].opt()],
            outs=[cc_out[j * NCORES * DH:(j + 1) * NCORES * DH, :].opt()])
        if after_stripe is not None:
            after_stripe(j)


def _qkvT_proj_groups(nc, ps, xT, w_sb, b_sb, outT, tag):
    """Thunks emitting outT[:, pair, stripe] = w.T @ xT + b (transposed)."""
    def mk(pair, jq):
        def emit():
            psq = ps.tile([128, QS], F32, name=f"psq_{tag}", tag=f"psq_{tag}",
                          bufs=2)
            for dc in range(NDC):
                nc.tensor.matmul(
                    psq[:],
                    w_sb[:, dc, pair * 128:(pair + 1) * 128],
                    xT[:, dc, jq * QS:(jq + 1) * QS],
                    start=(dc == 0), stop=(dc == NDC - 1))
            nc.scalar.activation(outT[:, pair, jq * QS:(jq + 1) * QS], psq[:],
                                 AF.Identity, bias=b_sb[:, pair:pair + 1])
        return emit
    return [mk(pair, jq) for pair in range(2) for jq in range(NQS)]


def _v_proj_groups(nc, ps, xT, wv_sb, bv_bc, Vaug, tag):
    """Thunks emitting Vaug[:, sb_i, h, 0:64] = xT.T @ wv + bv (natural)."""
    def mk(sb_i):
        def emit():
            psv = ps.tile([128, DH], F32, name=f"psv_{tag}", tag=f"psq_{tag}",
                          bufs=2)
            for dc in range(NDC):
                nc.tensor.matmul(
                    psv[:],
                    xT[:, dc, sb_i * KC:(sb_i + 1) * KC],
                    wv_sb[:, dc, :],
                    start=(dc == 0), stop=(dc == NDC - 1))
            nc.vector.tensor_tensor(
                out=Vaug[:, sb_i, :, 0:64],
                in0=psv[:].rearrange("p (h d) -> p h d", h=HPC),
                in1=bv_bc[:].rearrange("p (h d) -> p h d", h=HPC),
                op=OP.add)
        return emit
    return [mk(sb_i) for sb_i in range(NKC)]


def _layernorm_T(nc, sb, ps, xraw, g_sb, b_sb, outT, ones_col, ones_sb,
                 eps_sb, tag, out_res=None):
    """LayerNorm over d (partition-chunked) in transposed layout.
    xraw: [128, 8, 512] f32r. Stats via M=1 matmuls, broadcast via K=1."""
    pssum = ps.tile([1, Q4], F32, name=f"pssum_{tag}", tag="pssum", bufs=1)
    pssq = ps.tile([1, Q4], F32, name=f"pssq_{tag}", tag="pssq", bufs=1)
    for m in range(NDC):
        nc.tensor.matmul(pssum[:], ones_col[:, 0:1], xraw[:, m, :],
                         start=(m == 0), stop=(m == NDC - 1))
    for m in range(NDC):
        xsq = sb.tile([128, Q4], F32R, name="xsq", tag="xsq", bufs=2)
        nc.vector.tensor_mul(xsq[:], xraw[:, m, :], xraw[:, m, :])
        nc.tensor.matmul(pssq[:], ones_col[:, 0:1], xsq[:],
                         start=(m == 0), stop=(m == NDC - 1))
    mu = sb.tile([1, Q4], F32R, name="mu", tag="mu", bufs=2)
    nc.scalar.activation(mu[:], pssum[:], AF.Copy, scale=1.0 / D)
    msq = sb.tile([1, Q4], F32, name="msq", tag="msq", bufs=2)
    nc.scalar.activation(msq[:], pssq[:], AF.Copy, scale=1.0 / D)
    musq = sb.tile([1, Q4], F32, name="musq", tag="musq", bufs=2)
    nc.vector.tensor_mul(musq[:], mu[:], mu[:])
    var = sb.tile([1, Q4], F32, name="var", tag="var", bufs=2)
    nc.vector.tensor_tensor(out=var[:], in0=msq[:], in1=musq[:],
                            op=OP.subtract)
    sd = sb.tile([1, Q4], F32, name="sd", tag="sd", bufs=2)
    nc.scalar.activation(sd[:], var[:], AF.Sqrt, bias=eps_sb[:])
    rstd = sb.tile([1, Q4], F32R, name="rstd", tag="rstd", bufs=2)
    with nc.allow_low_precision(reason="LN rstd recip"):
        nc.vector.reciprocal(rstd[:], sd[:])
    psmu = ps.tile([128, Q4], F32, name=f"psmu_{tag}", tag="psmu", bufs=1)
    psrs = ps.tile([128, Q4], F32, name=f"psrs_{tag}", tag="psrs", bufs=1)
    nc.tensor.matmul(psmu[:], ones_sb[0:1, :], mu[:], start=True, stop=True)
    nc.tensor.matmul(psrs[:], ones_sb[0:1, :], rstd[:], start=True, stop=True)
    mu_b = sb.tile([128, Q4], F32, name="mu_b", tag="mu_b", bufs=2)
    rs_b = sb.tile([128, Q4], F32, name="rs_b", tag="rs_b", bufs=2)
    nc.scalar.copy(mu_b[:], psmu[:])
    nc.scalar.copy(rs_b[:], psrs[:])
    for m in range(NDC):
        tmp = sb.tile([128, Q4], F32, name="lntmp", tag="lntmp", bufs=2)
        nc.vector.tensor_tensor(out=tmp[:], in0=xraw[:, m, :], in1=mu_b[:],
                                op=OP.subtract)
        nc.vector.tensor_mul(tmp[:], tmp[:], rs_b[:])
        nc.vector.tensor_scalar(
            out=outT[:, m, :], in0=tmp[:],
            scalar1=g_sb[:, m:m + 1], scalar2=b_sb[:, m:m + 1],
            op0=OP.mult, op1=OP.add)
        if out_res is not None:
            nc.vector.tensor_scalar(
                out=out_res[:, m, :], in0=tmp[:],
                scalar1=g_sb[:, m:m + 1], scalar2=b_sb[:, m:m + 1],
                op0=OP.mult, op1=OP.add)


def _out_proj_ln(nc, sb, ln_sb_pool, ps, ln_ps, wo_d, bo_sb, cc_out, row_off,
                 res_src, g_sb, b_sb, outT, out_res, ones_col, ones_sb,
                 eps_sb, tag):
    """Out-projection (from gathered ctx, my quarter) + residual + LN."""
    gctx = sb.tile([128, NDC, Q4], BF16, name=f"gctx_{tag}", tag=f"gctx_{tag}")
    for c in range(NDC):
        nc.sync.dma_start(gctx[:, c, :],
                          cc_out[bass.ds(row_off + 128 * c, 128), :])
    xraw = sb.tile([128, NDC, Q4], F32R, name=f"xraw_{tag}", tag=f"xraw_{tag}")
    for m in range(NDC):
        wom = sb.tile([128, NDC, 128], BF16, name=f"wom_{tag}",
                      tag=f"wom_{tag}", bufs=2)
        nc.sync.dma_start(
            wom[:],
            wo_d[:, m * 128:(m + 1) * 128].rearrange("(c p) n -> p c n",
                                                     p=128))
        pso = ps.tile([128, Q4], F32, name=f"pso_{tag}", tag="pso", bufs=2)
        for c in range(NDC):
            nc.tensor.matmul(pso[:], wom[:, c, :], gctx[:, c, :],
                             start=(c == 0), stop=(c == NDC - 1))
        t_sb = sb.tile([128, Q4], F32, name=f"tsb_{tag}", tag=f"tsb_{tag}",
                       bufs=2)
        nc.scalar.activation(t_sb[:], pso[:], AF.Identity,
                             bias=bo_sb[:, m:m + 1])
        nc.vector.tensor_tensor(out=xraw[:, m, :], in0=t_sb[:],
                                in1=res_src(m), op=OP.add)
    _layernorm_T(nc, ln_sb_pool, ln_ps, xraw, g_sb, b_sb, outT, ones_col,
                 ones_sb, eps_sb, tag, out_res=out_res)


def build(modes_sa, n_mask_sa, modes_ca, n_mask_ca, mask_index_sa,
          mask_index_ca):
    nc = bacc.Bacc("TRN2", num_devices=NCORES)

    def dram_in(name, shape, dt=BF16):
        return nc.dram_tensor(name, shape, dt, kind="ExternalInput")

    tgtT_d = dram_in("tgtT", [D, S])
    tgtTq_d = dram_in("tgtTq", [D, Q4], F32)
    memT_d = dram_in("memT", [D, S])
    w_d = {}
    for a in ("sa", "ca"):
        for w in ("wq", "wk", "wv"):
            w_d[f"{w}_{a}"] = dram_in(f"{w}_{a}", [D, DH])
        w_d[f"wo_{a}"] = dram_in(f"wo_{a}", [D, D])
    w1_d = dram_in("w1", [D, DFF])
    w2_d = dram_in("w2", [DFF, D])
    b_d = {}
    for a in ("sa", "ca"):
        for bn in ("bq", "bk", "bv"):
            b_d[f"{bn}_{a}"] = dram_in(f"{bn}_{a}", [DH], F32)
        b_d[f"bo_{a}"] = dram_in(f"bo_{a}", [D], F32)
    b1_d = dram_in("b1", [DFF], F32)
    b2_d = dram_in("b2", [D], F32)
    ln_d = {}
    for i in (1, 2, 3):
        ln_d[f"g{i}"] = dram_in(f"ln{i}_g", [D], F32)
        ln_d[f"b{i}"] = dram_in(f"ln{i}_b", [D], F32)
    ones_d = dram_in("ones", [128, 128], F32R)
    ones_h_d = dram_in("ones_h", [128, 128], BF16)
    mask_sa_d = dram_in("mask_sa", [max(n_mask_sa, 1), KC, QS], BF16)
    mask_ca_d = dram_in("mask_ca", [max(n_mask_ca, 1), KC, QS], BF16)
    out_d = nc.dram_tensor("out", [D, Q4], F32, kind="ExternalOutput")

    with tile.TileContext(nc) as tc:
        with (
            tc.tile_pool(name="persist", bufs=1) as persist,
            tc.tile_pool(name="dram", bufs=1, space="DRAM") as dram,
        ):
            cc_in_sa = dram.tile([NQS, DH, Q4], BF16)
            cc_out_sa = dram.tile([NQS * NCORES * DH, Q4], BF16)
            cc_in_x1 = dram.tile([D, Q4], BF16)
            cc_out_x1 = dram.tile([NCORES * D, Q4], BF16, addr_space="Shared")
            cc_in_ca = dram.tile([NQS, DH, Q4], BF16)
            cc_out_ca = dram.tile([NQS * NCORES * DH, Q4], BF16)

            # runtime offsets from the SPMD partition id: my (stripe==quarter,
            # batch-group) row block inside the per-stripe 8-rank AG outputs
            pid = nc.sync.partition_id()
            ctx_row = (pid % TP) * (NCORES * DH) + (pid // TP) * (TP * DH)
            x1_boff = (pid // TP) * (TP * D)

            ones_sb = persist.tile([1, 128], F32R)
            nc.sync.dma_start(ones_sb[:], ones_d[0:1, :])
            ones_col = persist.tile([128, 1], F32R)
            nc.sync.dma_start(ones_col[:], ones_d[:, 0:1])
            eps_sb = persist.tile([1, 1], F32)
            nc.vector.memset(eps_sb[:], EPS)

            def vec_sb(d, n):
                t = persist.tile([128, n // 128], F32, name=f"v_{d.name}",
                                 tag=f"v_{d.name}")
                nc.sync.dma_start(t[:], d[:].rearrange("(c p) -> p c", p=128))
                return t

            bsb = {k: vec_sb(v, v.shape[0]) for k, v in b_d.items()}
            b1_sb = vec_sb(b1_d, DFF)
            b2_sb = vec_sb(b2_d, D)
            ln_sb = {k: vec_sb(v, D) for k, v in ln_d.items()}

            def bcast_sb(d, n):
                t = persist.tile([128, n], F32, name=f"bc_{d.name}",
                                 tag=f"bc_{d.name}")
                nc.gpsimd.dma_start(
                    out=t[:],
                    in_=bass.AP(tensor=d, offset=0, ap=[[0, 128], [1, n]]))
                return t

            bv_sa_bc = bcast_sb(b_d["bv_sa"], DH)
            bv_ca_bc = bcast_sb(b_d["bv_ca"], DH)

            with tc.tile_pool(name="keep", bufs=1) as keep:
                x1res = keep.tile([128, NDC, Q4], F32)
                x2T = keep.tile([128, NDC, Q4], BF16)
                x2res = keep.tile([128, NDC, Q4], F32)
                QT_ca = keep.tile([128, 2, S], BF16)
                KT_ca = keep.tile([128, 2, S], BF16)
                Vaug_ca = keep.tile([128, NKC, HPC, 65], BF16)
                nc.sync.dma_start(
                    Vaug_ca[:, :, :, 64:65],
                    ones_h_d[:, 0:64].rearrange("p (a b c) -> p a b c",
                                                a=NKC, b=HPC))

                with tc.tile_pool(name="pA", bufs=1) as pA:
                    # ---------- Phase 0/1: loads + self QKV ----------
                    memT = pA.tile([128, NDC, S], BF16)
                    nc.sync.dma_start(
                        memT[:],
                        memT_d[:].rearrange("(c p) s -> p c s", p=128))
                    w_sb = {}
                    for wn in ("wq_ca", "wk_ca", "wv_ca"):
                        t = pA.tile([128, NDC, DH], BF16, name=f"w_{wn}",
                                    tag=f"w_{wn}")
                        nc.sync.dma_start(
                            t[:],
                            w_d[wn][:].rearrange("(c p) n -> p c n", p=128))
                        w_sb[wn] = t

                    with tc.tile_pool(name="pC", bufs=1) as pC:
                        QT_sa = pC.tile([128, 2, S], BF16)
                        KT_sa = pC.tile([128, 2, S], BF16)
                        Vaug_sa = pC.tile([128, NKC, HPC, 65], BF16)
                        nc.sync.dma_start(
                            Vaug_sa[:, :, :, 64:65],
                            ones_h_d[:, 0:64].rearrange(
                                "p (a b c) -> p a b c", a=NKC, b=HPC))

                        with (
                            tc.tile_pool(name="pB", bufs=1) as pB,
                            tc.tile_pool(name="proj_ps", bufs=1,
                                         space="PSUM") as proj_ps,
                        ):
                            tgtT = pB.tile([128, NDC, S], BF16)
                            nc.sync.dma_start(
                                tgtT[:],
                                tgtT_d[:].rearrange("(c p) s -> p c s",
                                                    p=128))
                            for wn in ("wq_sa", "wk_sa", "wv_sa"):
                                t = pB.tile([128, NDC, DH], BF16,
                                            name=f"w_{wn}", tag=f"w_{wn}")
                                nc.sync.dma_start(
                                    t[:],
                                    w_d[wn][:].rearrange("(c p) n -> p c n",
                                                         p=128))
                                w_sb[wn] = t
                            for g in (_qkvT_proj_groups(nc, proj_ps, tgtT,
                                                        w_sb["wq_sa"],
                                                        bsb["bq_sa"], QT_sa,
                                                        "p1")
                                      + _qkvT_proj_groups(nc, proj_ps, tgtT,
                                                          w_sb["wk_sa"],
                                                          bsb["bk_sa"],
                                                          KT_sa, "p1")
                                      + _v_proj_groups(nc, proj_ps, tgtT,
                                                       w_sb["wv_sa"],
                                                       bv_sa_bc, Vaug_sa,
                                                       "p1")):
                                g()

                        # ------ Phase 2: self attention + woven cross-K/V
                        # projections + per-stripe ctx AllGathers ------
                        with (
                            tc.tile_pool(name="attn_sb", bufs=1) as attn_sb,
                            tc.tile_pool(name="attn_ps", bufs=1,
                                         space="PSUM") as attn_ps,
                            tc.tile_pool(name="kv_ps", bufs=1,
                                         space="PSUM") as kv_ps,
                        ):
                            kv_groups = (
                                _qkvT_proj_groups(nc, kv_ps, memT,
                                                  w_sb["wk_ca"],
                                                  bsb["bk_ca"], KT_ca, "kv")
                                + _v_proj_groups(nc, kv_ps, memT,
                                                 w_sb["wv_ca"], bv_ca_bc,
                                                 Vaug_ca, "kv"))
                            n_per = (len(kv_groups) + NQS - 1) // NQS

                            def after_stripe(j):
                                for g in kv_groups[j * n_per:(j + 1) * n_per]:
                                    g()

                            _attention(nc, tc, attn_sb, attn_ps, QT_sa,
                                       KT_sa, Vaug_sa, modes_sa,
                                       mask_index_sa, mask_sa_d, cc_in_sa,
                                       cc_out_sa, ones_sb, 1, after_stripe)

                    # ---------- Phase 4: self out-proj + LN1 ----------
                    with (
                        tc.tile_pool(name="p4_sb", bufs=1) as p4sb,
                        tc.tile_pool(name="ln_sb1", bufs=1) as ln_sb1,
                        tc.tile_pool(name="p4_ps", bufs=1,
                                     space="PSUM") as p4ps,
                        tc.tile_pool(name="ln_ps1", bufs=1,
                                     space="PSUM") as ln_ps1,
                    ):
                        tgtTq = p4sb.tile([128, NDC, Q4], F32)
                        nc.sync.dma_start(
                            tgtTq[:],
                            tgtTq_d[:].rearrange("(c p) q -> p c q", p=128))
                        x1T = p4sb.tile([128, NDC, Q4], BF16)
                        _out_proj_ln(nc, p4sb, ln_sb1, p4ps, ln_ps1,
                                     w_d["wo_sa"], bsb["bo_sa"], cc_out_sa,
                                     ctx_row, lambda m: tgtTq[:, m, :],
                                     ln_sb["g1"], ln_sb["b1"], x1T, x1res,
                                     ones_col, ones_sb, eps_sb, "ln1")
                        nc.sync.dma_start(
                            cc_in_x1[:].rearrange("(c p) q -> p c q", p=128),
                            x1T[:])

                    nc.gpsimd.collective_compute(
                        "AllGather", OP.bypass, replica_groups=ALLR,
                        ins=[cc_in_x1[:].opt()], outs=[cc_out_x1[:].opt()])

                    # ---------- Phase 5: cross Q projection ----------
                    with (
                        tc.tile_pool(name="p5_sb", bufs=2) as p5sb,
                        tc.tile_pool(name="p5_ps", bufs=1,
                                     space="PSUM") as p5ps,
                    ):
                        for jq in range(NQS):
                            x1f = p5sb.tile([128, NDC, QS], BF16, name="x1f",
                                            tag="x1f")
                            nc.sync.dma_start(
                                x1f[:],
                                cc_out_x1[bass.ds(x1_boff + D * jq, D), :]
                                .rearrange("(c p) q -> p c q", p=128))
                            for pair in range(2):
                                psq = p5ps.tile([128, QS], F32, name="psq5",
                                                tag="psq5", bufs=2)
                                for dc in range(NDC):
                                    nc.tensor.matmul(
                                        psq[:],
                                        w_sb["wq_ca"][
                                            :, dc,
                                            pair * 128:(pair + 1) * 128],
                                        x1f[:, dc, :],
                                        start=(dc == 0),
                                        stop=(dc == NDC - 1))
                                nc.scalar.activation(
                                    QT_ca[:, pair, jq * QS:(jq + 1) * QS],
                                    psq[:], AF.Identity,
                                    bias=bsb["bq_ca"][:, pair:pair + 1])

                # ---------- Phase 6: cross attention ----------
                with (
                    tc.tile_pool(name="attn_sb2", bufs=1) as attn_sb2,
                    tc.tile_pool(name="attn_ps2", bufs=1,
                                 space="PSUM") as attn_ps2,
                ):
                    _attention(nc, tc, attn_sb2, attn_ps2, QT_ca, KT_ca,
                               Vaug_ca, modes_ca, mask_index_ca, mask_ca_d,
                               cc_in_ca, cc_out_ca, ones_sb, 2, None)

                # ---------- Phase 7: cross out-proj + LN2 ----------
                with (
                    tc.tile_pool(name="p7_sb", bufs=1) as p7sb,
                    tc.tile_pool(name="ln_sb2", bufs=1) as ln_sb2,
                    tc.tile_pool(name="p7_ps", bufs=1, space="PSUM") as p7ps,
                    tc.tile_pool(name="ln_ps2", bufs=1,
                                 space="PSUM") as ln_ps2,
                ):
                    _out_proj_ln(nc, p7sb, ln_sb2, p7ps, ln_ps2,
                                 w_d["wo_ca"], bsb["bo_ca"], cc_out_ca,
                                 ctx_row, lambda m: x1res[:, m, :],
                                 ln_sb["g2"], ln_sb["b2"], x2T, x2res,
                                 ones_col, ones_sb, eps_sb, "ln2")

                # ---------- Phase 8: FFN + LN3 ----------
                with (
                    tc.tile_pool(name="p8_h", bufs=1) as p8h,
                    tc.tile_pool(name="p8_w", bufs=3) as p8w,
                    tc.tile_pool(name="p8_sb", bufs=1) as p8sb,
                    tc.tile_pool(name="ln_sb3", bufs=1) as ln_sb3,
                    tc.tile_pool(name="p8_ps", bufs=1, space="PSUM") as p8ps,
                    tc.tile_pool(name="ln_ps3", bufs=1,
                                 space="PSUM") as ln_ps3,
                ):
                    hT = p8h.tile([128, NFC, Q4], BF16)
                    for f in range(NFC):
                        w1f = p8w.tile([128, NDC, 128], BF16, name="w1f",
                                       tag="w1f")
                        nc.sync.dma_start(
                            w1f[:],
                            w1_d[:, f * 128:(f + 1) * 128]
                            .rearrange("(c p) n -> p c n", p=128))
                        psh = p8ps.tile([128, Q4], F32, name="psh", tag="psh",
                                        bufs=2)
                        for m in range(NDC):
                            nc.tensor.matmul(psh[:], w1f[:, m, :],
                                             x2T[:, m, :],
                                             start=(m == 0),
                                             stop=(m == NDC - 1))
                        nc.scalar.activation(hT[:, f, :], psh[:], AF.Relu,
                                             bias=b1_sb[:, f:f + 1])
                    x3raw = p8sb.tile([128, NDC, Q4], F32R)
                    for m in range(NDC):
                        w2m = p8w.tile([128, NFC, 128], BF16, name="w2m",
                                       tag="w2m", bufs=2)
                        nc.sync.dma_start(
                            w2m[:],
                            w2_d[:, m * 128:(m + 1) * 128]
                            .rearrange("(c p) n -> p c n", p=128))
                        psf = p8ps.tile([128, Q4], F32, name="psf", tag="psf",
                                        bufs=2)
                        for f in range(NFC):
                            nc.tensor.matmul(psf[:], w2m[:, f, :],
                                             hT[:, f, :],
                                             start=(f == 0),
                                             stop=(f == NFC - 1))
                        t_sb = p8sb.tile([128, Q4], F32, name="t_sb8",
                                         tag="t_sb8", bufs=2)
                        nc.scalar.activation(t_sb[:], psf[:], AF.Identity,
                                             bias=b2_sb[:, m:m + 1])
                        nc.vector.tensor_tensor(out=x3raw[:, m, :],
                                                in0=t_sb[:],
                                                in1=x2res[:, m, :],
                                                op=OP.add)

                    x3T = p8sb.tile([128, NDC, Q4], F32)
                    _layernorm_T(nc, ln_sb3, ln_ps3, x3raw, ln_sb["g3"],
                                 ln_sb["b3"], x3T, ones_col, ones_sb, eps_sb,
                                 "ln3")
                    nc.sync.dma_start(
                        out_d[:].rearrange("(c p) q -> p c q", p=128), x3T[:])

    nc.finalize()
    return nc


_CACHE = {}


def _get_kernel(tgt_mask, memory_mask):
    modes_sa, tiles_sa, idx_sa = _analyze_mask(np.asarray(tgt_mask))
    modes_ca, tiles_ca, idx_ca = _analyze_mask(np.asarray(memory_mask))
    key = (tuple(map(tuple, modes_sa)), tuple(map(tuple, modes_ca)))
    if key not in _CACHE:
        nc = build(modes_sa, len(idx_sa), modes_ca, len(idx_ca), idx_sa,
                   idx_ca)
        _CACHE[key] = nc
    return _CACHE[key], tiles_sa, tiles_ca


def _run(inputs, trace=False):
    tgt = np.asarray(inputs["tgt"], np.float32)
    memory = np.asarray(inputs["memory"], np.float32)
    nc, tiles_sa, tiles_ca = _get_kernel(inputs["tgt_mask"],
                                         inputs["memory_mask"])

    f32 = lambda x: np.ascontiguousarray(np.asarray(x), dtype=np.float32)
    bf = lambda x: np.ascontiguousarray(
        np.asarray(x, dtype=np.float32).astype(ml_dtypes.bfloat16))
    ones128 = np.ones((128, 128), np.float32)
    shared = {
        "wo_sa": bf(inputs["sa_wo"]), "bo_sa": f32(inputs["sa_bo"]),
        "wo_ca": bf(inputs["ca_wo"]), "bo_ca": f32(inputs["ca_bo"]),
        "w1": bf(inputs["ff_w1"]), "b1": f32(inputs["ff_b1"]),
        "w2": bf(inputs["ff_w2"]), "b2": f32(inputs["ff_b2"]),
        "ln1_g": f32(inputs["ln1_g"]), "ln1_b": f32(inputs["ln1_b"]),
        "ln2_g": f32(inputs["ln2_g"]), "ln2_b": f32(inputs["ln2_b"]),
        "ln3_g": f32(inputs["ln3_g"]), "ln3_b": f32(inputs["ln3_b"]),
        "ones": ones128, "ones_h": ones128.astype(ml_dtypes.bfloat16),
        "mask_sa": bf(tiles_sa), "mask_ca": bf(tiles_ca),
    }
    scale = 1.0 / np.sqrt(DK)
    in_maps = []
    for cid in range(NCORES):
        b, g = cid // TP, cid % TP
        hs = slice(g * DH, (g + 1) * DH)
        m = dict(shared)
        m["tgtT"] = bf(tgt[:, b, :].T)
        m["tgtTq"] = f32(tgt[g * Q4:(g + 1) * Q4, b, :].T)
        m["memT"] = bf(memory[:, b, :].T)
        m["wq_sa"] = bf(np.asarray(inputs["sa_wq"])[:, hs] * scale)
        m["bq_sa"] = f32(np.asarray(inputs["sa_bq"])[hs] * scale)
        m["wk_sa"] = bf(np.asarray(inputs["sa_wk"])[:, hs])
        m["bk_sa"] = f32(np.asarray(inputs["sa_bk"])[hs])
        m["wv_sa"] = bf(np.asarray(inputs["sa_wv"])[:, hs])
        m["bv_sa"] = f32(np.asarray(inputs["sa_bv"])[hs])
        m["wq_ca"] = bf(np.asarray(inputs["ca_wq"])[:, hs] * scale)
        m["bq_ca"] = f32(np.asarray(inputs["ca_bq"])[hs] * scale)
        m["wk_ca"] = bf(np.asarray(inputs["ca_wk"])[:, hs])
        m["bk_ca"] = f32(np.asarray(inputs["ca_bk"])[hs])
        m["wv_ca"] = bf(np.asarray(inputs["ca_wv"])[:, hs])
        m["bv_ca"] = f32(np.asarray(inputs["ca_bv"])[hs])
        in_maps.append(m)

    res = run_bass_kernel_spmd(nc, in_maps, core_ids=list(range(NCORES)),
                               trace=trace)
    out = np.empty((S, B, D), np.float32)
    for cid in range(NCORES):
        b, g = cid // TP, cid % TP
        out[g * Q4:(g + 1) * Q4, b, :] = res.results[cid]["out"].T
    return out, res


def kernel(**inputs):
    out, _ = _run(inputs, trace=False)
    return out
